# revision 1
# baseline (speedup 1.0000x reference)
"""Trainium2 Bass kernel for MAB (multihead attention block) — nn_MAB_48412871360901.

Data-parallel over batch: 16 batches -> 8 NeuronCores, 2 batches/core.
Per core, per batch (all matmuls bf16 with fp32 PSUM accumulation):
  P1  Q,K loaded natural, cast bf16, PE-transposed -> QT/KT  [dv, nq] layout
  P2  projections qT = Wq^T QT, kT = Wk^T KT (transposed layout), v = K Wv (natural)
  P3  per (head-pair, q-chunk): S^T = k^T.T q^T (row-packed 2 heads),
      exp on ACT (scale 1/sqrt(512) fused), softmax denominators Z via
      ones-matmuls, PV U^T = v^T expS^T (col-packed 2 heads),
      X^T = U^T * (1/Z)bcast + qT   (residual uses post-projection q)
  P4  LN0 in transposed layout: stats via ones-matmuls over partitions,
      rstd = exp(-0.5 ln(var+eps)) on ACT, normalize with PE-broadcast tiles
  P5  M = Xn Wo (natural out) + identity-fold transpose of Xn; relu; residual;
      LN1 natural (bn_stats); DMA out fp32.
"""

import sys
from contextlib import ExitStack
import numpy as np
import ml_dtypes

for _p in ("/opt/trn_rl_repo", "/root/.axon_site/_ro/trn_rl_repo"):
    if _p not in sys.path:
        sys.path.insert(0, _p)

import concourse.bacc as bacc
import concourse.mybir as mybir
import concourse.tile as tile
from concourse.bass_utils import run_bass_kernel_spmd

BF16 = mybir.dt.bfloat16
F32 = mybir.dt.float32
NBF = ml_dtypes.bfloat16
AF = mybir.ActivationFunctionType
OP = mybir.AluOpType

B, NQ, NK = 16, 1024, 1024
D = 512
H = 8
N_CORES = 8
BL = B // N_CORES          # batches per core
EPS = 1e-5
SCALE = 1.0 / np.sqrt(512.0)

_cache = {}


class _Ctx:
    pass


def _setup_consts(nc, cx, cst, flags):
    (bq_nz, bk_nz, bv_nz, bo_nz, ln0_aff, ln1_aff) = flags

    def din(name, shape, dt=BF16):
        return nc.dram_tensor(name, list(shape), dt, kind="ExternalInput").ap()

    def ldc(name, dshape, shape, rearr=None):
        d = din(name, dshape)
        t = cst.tile(list(shape), BF16, tag=name)
        nc.sync.dma_start(out=t, in_=d if rearr is None else d.rearrange(rearr, p=128))
        return t

    def ldf(name, shape):
        d = din(name, shape, F32)
        t = cst.tile(list(shape), F32, tag=name)
        nc.sync.dma_start(out=t, in_=d)
        return t

    cx.w_q = ldc("Wqb", (D, D), (128, 4, D), "(kt p) c -> p kt c")
    cx.w_k = ldc("Wkb", (D, D), (128, 4, D), "(kt p) c -> p kt c")
    cx.w_v = ldc("Wvb", (D, D), (128, 4, D), "(kt p) c -> p kt c")
    cx.w_o = ldc("Wob", (D, D), (128, 4, D), "(kt p) c -> p kt c")
    cx.i512 = ldc("I512b", (D, D), (128, 4, D), "(kt p) c -> p kt c")
    cx.id128 = ldc("I128b", (128, 128), (128, 128))
    cx.onesc = ldc("onesc", (128, 1), (128, 1))
    cx.onesr = ldc("onesr", (1, 128), (1, 128))
    cx.mk2 = ldc("mk2", (2, 128), (2, 128))
    cx.epsP = cst.tile([128, 1], F32, tag="epsP"); nc.vector.memset(cx.epsP, EPS)
    cx.eps1 = cst.tile([1, 1], F32, tag="eps1"); nc.vector.memset(cx.eps1, EPS)
    cx.bq4 = ldf("bq4", (128, 4)) if bq_nz else None
    cx.bk4 = ldf("bk4", (128, 4)) if bk_nz else None
    cx.bvb = ldf("bvb", (128, D)) if bv_nz else None
    cx.bob = ldf("bob", (128, D)) if bo_nz else None
    cx.g04 = ldf("g04", (128, 4)) if ln0_aff else None
    cx.b04 = ldf("b04", (128, 4)) if ln0_aff else None
    cx.g1b = ldf("g1b", (128, D)) if ln1_aff else None
    cx.b1b = ldf("b1b", (128, D)) if ln1_aff else None


def _p1_transpose(nc, cx, rb, src_dram, tag):
    dst = cx.p_qkt.tile([128, 4, NQ], BF16, tag=tag)
    for i in range(8):
        nat = cx.p_nat.tile([128, D], F32)
        nc.sync.dma_start(out=nat, in_=src_dram[rb + 128 * i: rb + 128 * (i + 1), :])
        natb = cx.p_natb.tile([128, D], BF16)
        nc.vector.tensor_copy(out=natb, in_=nat)
        tp = cx.ps_half.tile([128, D], BF16, tag="h")
        for j in range(4):
            nc.tensor.transpose(tp[:, 128 * j:128 * (j + 1)],
                                natb[:, 128 * j:128 * (j + 1)], cx.id128)
        nc.vector.tensor_copy(
            out=dst[:, :, 128 * i:128 * (i + 1)],
            in_=tp.rearrange("p (j c) -> p j c", j=4))
    return dst


def _p2_proj(nc, cx, QT, KT):
    qT = cx.p_proj.tile([128, 4, NQ], BF16, tag="qT")
    kT = cx.p_proj.tile([128, 4, NQ], BF16, tag="kT")
    vT = cx.p_proj.tile([128, 8, D], BF16, tag="vT")

    for dst, w, srcT, bias in ((qT, cx.w_q, QT, cx.bq4), (kT, cx.w_k, KT, cx.bk4)):
        for dvt in range(4):
            for qc in range(2):
                pp = cx.ps_half.tile([128, D], F32, tag="h")
                for kt in range(4):
                    nc.tensor.matmul(
                        pp, lhsT=w[:, kt, 128 * dvt:128 * (dvt + 1)],
                        rhs=srcT[:, kt, 512 * qc:512 * (qc + 1)],
                        start=(kt == 0), stop=(kt == 3))
                o = dst[:, dvt, 512 * qc:512 * (qc + 1)]
                if bias is not None:
                    nc.vector.tensor_scalar_add(out=o, in0=pp, scalar1=bias[:, dvt:dvt + 1])
                else:
                    nc.vector.tensor_copy(out=o, in_=pp)
    for nkt in range(8):
        pp = cx.ps_half.tile([128, D], F32, tag="h")
        for kt in range(4):
            nc.tensor.matmul(pp, lhsT=KT[:, kt, 128 * nkt:128 * (nkt + 1)],
                             rhs=cx.w_v[:, kt, :], start=(kt == 0), stop=(kt == 3))
        if cx.bvb is not None:
            nc.vector.scalar_tensor_tensor(out=vT[:, nkt, :], in0=pp, scalar=0.0,
                                           in1=cx.bvb, op0=OP.add, op1=OP.add)
        else:
            nc.vector.tensor_copy(out=vT[:, nkt, :], in_=pp)
    return qT, kT, vT


def _p3_attn_unit(nc, cx, qT, kT, vT, XT, SQ, hp, qc):
    ps_u = cx.ps_pv.tile([128, D], F32, tag="u")
    ps_z = cx.ps_half.tile([128, D], F32, tag="h")
    for kt in range(8):
        ps_s = cx.ps_wide.tile([128, 2 * D], F32, tag="w")
        nc.tensor.matmul(
            ps_s[:, 0:D],
            lhsT=kT[0:64, hp, 128 * kt:128 * (kt + 1)],
            rhs=qT[0:64, hp, 512 * qc:512 * (qc + 1)],
            start=True, stop=True, tile_position=(0, 0))
        nc.tensor.matmul(
            ps_s[:, D:2 * D],
            lhsT=kT[64:128, hp, 128 * kt:128 * (kt + 1)],
            rhs=qT[64:128, hp, 512 * qc:512 * (qc + 1)],
            start=True, stop=True, tile_position=(64, 0))
        ex = cx.p_ex.tile([128, 2 * D], BF16)
        nc.scalar.activation(out=ex, in_=ps_s, func=AF.Exp, scale=SCALE)
        nc.tensor.matmul(ps_z[0:1, :], lhsT=cx.onesc, rhs=ex[:, 0:D],
                         start=(kt == 0), stop=(kt == 7), tile_position=(0, 0))
        nc.tensor.matmul(ps_z[32:33, :], lhsT=cx.onesc, rhs=ex[:, D:2 * D],
                         start=(kt == 0), stop=(kt == 7), tile_position=(0, 32))
        nc.tensor.matmul(ps_u[0:64, :],
                         lhsT=vT[:, kt, 128 * hp:128 * hp + 64],
                         rhs=ex[:, 0:D],
                         start=(kt == 0), stop=(kt == 7), tile_position=(0, 0))
        nc.tensor.matmul(ps_u[64:128, :],
                         lhsT=vT[:, kt, 128 * hp + 64:128 * (hp + 1)],
                         rhs=ex[:, D:2 * D],
                         start=(kt == 0), stop=(kt == 7), tile_position=(0, 64))
    rz = cx.p_sml.tile([64, D], BF16, tag="rz")
    with nc.allow_low_precision(reason="softmax 1/Z in bf16 is consistent with bf16 probs"):
        nc.vector.reciprocal(out=rz, in_=ps_z[0:64, :])
    rz2 = cx.p_sml.tile([2, D], BF16, tag="rz2")
    nc.sync.dma_start(out=rz2[0:1, :], in_=rz[0:1, :])
    nc.sync.dma_start(out=rz2[1:2, :], in_=rz[32:33, :])
    ps_rz = cx.ps_half.tile([128, D], F32, tag="h")
    nc.tensor.matmul(ps_rz, lhsT=cx.mk2, rhs=rz2, start=True, stop=True)
    u_bf = cx.p_ub.tile([128, D], BF16)
    nc.vector.tensor_copy(out=u_bf, in_=ps_u)
    t1 = cx.p_t1.tile([128, D], BF16, tag="t")
    nc.vector.tensor_tensor(out=t1, in0=u_bf, in1=ps_rz, op=OP.mult)
    xs = XT[:, hp, 512 * qc:512 * (qc + 1)]
    nc.vector.tensor_tensor(out=xs, in0=t1,
                            in1=qT[:, hp, 512 * qc:512 * (qc + 1)], op=OP.add)
    nc.vector.tensor_tensor(out=SQ[:, hp, 512 * qc:512 * (qc + 1)],
                            in0=xs, in1=xs, op=OP.mult)


def _p4_ln0(nc, cx, XT, SQ, ln0_aff):
    XnT = cx.p_xnt.tile([128, 4, NQ], BF16)
    for qc in range(2):
        ps_st = cx.ps_half.tile([128, D], F32, tag="h")
        for dvt in range(4):
            nc.tensor.matmul(ps_st[0:1, :], lhsT=cx.onesc,
                             rhs=XT[:, dvt, 512 * qc:512 * (qc + 1)],
                             start=(dvt == 0), stop=(dvt == 3), tile_position=(0, 0))
            nc.tensor.matmul(ps_st[32:33, :], lhsT=cx.onesc,
                             rhs=SQ[:, dvt, 512 * qc:512 * (qc + 1)],
                             start=(dvt == 0), stop=(dvt == 3), tile_position=(0, 32))
        mu = cx.p_sml.tile([1, D], F32, tag="mu")
        nc.vector.tensor_scalar_mul(out=mu, in0=ps_st[0:1, :], scalar1=1.0 / D)
        mu2 = cx.p_sml.tile([1, D], F32, tag="mu2")
        nc.vector.tensor_tensor(out=mu2, in0=mu, in1=mu, op=OP.mult)
        var = cx.p_sml.tile([1, D], F32, tag="var")
        nc.vector.scalar_tensor_tensor(out=var, in0=ps_st[32:33, :],
                                       scalar=1.0 / D, in1=mu2,
                                       op0=OP.mult, op1=OP.subtract)
        lnv = cx.p_sml.tile([1, D], F32, tag="lnv")
        nc.scalar.activation(out=lnv, in_=var, func=AF.Ln, bias=cx.eps1, scale=1.0)
        rstd = cx.p_sml.tile([1, D], BF16, tag="rstd")
        nc.scalar.activation(out=rstd, in_=lnv, func=AF.Exp, scale=-0.5)
        nmr = cx.p_sml.tile([1, D], BF16, tag="nmr")
        nc.vector.scalar_tensor_tensor(out=nmr, in0=mu, scalar=-1.0, in1=rstd,
                                       op0=OP.mult, op1=OP.mult)
        ps_b2 = cx.ps_wide.tile([128, 2 * D], F32, tag="w")
        nc.tensor.matmul(ps_b2[:, 0:D], lhsT=cx.onesr, rhs=rstd, start=True, stop=True)
        nc.tensor.matmul(ps_b2[:, D:2 * D], lhsT=cx.onesr, rhs=nmr, start=True, stop=True)
        for dvt in range(4):
            t2 = cx.p_t1.tile([128, D], BF16, tag="t")
            nc.vector.tensor_tensor(out=t2, in0=XT[:, dvt, 512 * qc:512 * (qc + 1)],
                                    in1=ps_b2[:, 0:D], op=OP.mult)
            xn = XnT[:, dvt, 512 * qc:512 * (qc + 1)]
            nc.vector.tensor_tensor(out=xn, in0=t2, in1=ps_b2[:, D:2 * D], op=OP.add)
            if ln0_aff:
                nc.vector.tensor_scalar(out=xn, in0=xn,
                                        scalar1=cx.g04[:, dvt:dvt + 1],
                                        scalar2=cx.b04[:, dvt:dvt + 1],
                                        op0=OP.mult, op1=OP.add)
    return XnT


def _p5_out(nc, cx, XnT, dOut, rb, ln1_aff):
    xpre_l, mv_l = [], []
    vars8 = cx.p_sml.tile([128, 8], F32, tag="vars8")
    for nqt in range(8):
        ps_m = cx.ps_wide.tile([128, 2 * D], F32, tag="w")
        for dvt in range(4):
            lb = XnT[:, dvt, 128 * nqt:128 * (nqt + 1)]
            nc.tensor.matmul(ps_m[:, 0:D], lhsT=lb, rhs=cx.w_o[:, dvt, :],
                             start=(dvt == 0), stop=(dvt == 3))
            nc.tensor.matmul(ps_m[:, D:2 * D], lhsT=lb, rhs=cx.i512[:, dvt, :],
                             start=(dvt == 0), stop=(dvt == 3))
        rl = cx.p_t1.tile([128, D], BF16, tag="t")
        if cx.bob is not None:
            tb = cx.p_t1.tile([128, D], BF16, tag="t")
            nc.vector.tensor_tensor(out=tb, in0=cx.bob, in1=ps_m[:, 0:D], op=OP.add)
            nc.vector.tensor_scalar_max(out=rl, in0=tb, scalar1=0.0)
        else:
            nc.vector.tensor_scalar_max(out=rl, in0=ps_m[:, 0:D], scalar1=0.0)
        xpre = cx.p_xp.tile([128, D], F32)
        nc.vector.tensor_tensor(out=xpre, in0=rl, in1=ps_m[:, D:2 * D], op=OP.add)
        bst = cx.p_mv.tile([128, 6], F32, tag="bst")
        nc.vector.bn_stats(out=bst, in_=xpre)
        mv = cx.p_mv.tile([128, 2], F32, tag="mv")
        nc.vector.bn_aggr(out=mv, in_=bst)
        nc.vector.tensor_copy(out=vars8[:, nqt:nqt + 1], in_=mv[:, 1:2])
        xpre_l.append(xpre); mv_l.append(mv)
    lnv8 = cx.p_sml.tile([128, 8], F32, tag="lnv8")
    nc.scalar.activation(out=lnv8, in_=vars8, func=AF.Ln, bias=cx.epsP, scale=1.0)
    rstd8 = cx.p_sml.tile([128, 8], F32, tag="rstd8")
    nc.scalar.activation(out=rstd8, in_=lnv8, func=AF.Exp, scale=-0.5)
    for nqt in range(8):
        ot = cx.p_out.tile([128, D], F32)
        nc.vector.tensor_scalar(out=ot, in0=xpre_l[nqt],
                                scalar1=mv_l[nqt][:, 0:1],
                                scalar2=rstd8[:, nqt:nqt + 1],
                                op0=OP.subtract, op1=OP.mult)
        if ln1_aff:
            nc.vector.tensor_tensor(out=ot, in0=ot, in1=cx.g1b, op=OP.mult)
            nc.vector.tensor_tensor(out=ot, in0=ot, in1=cx.b1b, op=OP.add)
        nc.sync.dma_start(out=dOut[rb + 128 * nqt: rb + 128 * (nqt + 1), :], in_=ot)


def _build(flags, repeat=1):
    (bq_nz, bk_nz, bv_nz, bo_nz, ln0_aff, ln1_aff) = flags
    nc = bacc.Bacc("TRN2", target_bir_lowering=False, debug=False,
                   num_devices=N_CORES)

    dQ = nc.dram_tensor("Qs", [BL * NQ, D], F32, kind="ExternalInput").ap()
    dK = nc.dram_tensor("Ks", [BL * NK, D], F32, kind="ExternalInput").ap()
    dOut = nc.dram_tensor("OUT", [BL * NQ, D], F32, kind="ExternalOutput").ap()

    cx = _Ctx()
    with ExitStack() as es:
        tc = es.enter_context(tile.TileContext(nc))
        ec = es.enter_context
        cst = ec(tc.tile_pool(name="cst", bufs=1))
        cx.p_qkt = ec(tc.tile_pool(name="qkt", bufs=1))
        cx.p_proj = ec(tc.tile_pool(name="proj", bufs=2))
        cx.p_xt = ec(tc.tile_pool(name="xt", bufs=1))
        cx.p_xnt = ec(tc.tile_pool(name="xnt", bufs=2))
        cx.p_nat = ec(tc.tile_pool(name="nat", bufs=2))
        cx.p_natb = ec(tc.tile_pool(name="natb", bufs=2))
        cx.p_ex = ec(tc.tile_pool(name="ex", bufs=3))
        cx.p_ub = ec(tc.tile_pool(name="ub", bufs=2))
        cx.p_t1 = ec(tc.tile_pool(name="t1", bufs=3))
        cx.p_xp = ec(tc.tile_pool(name="xp", bufs=9))
        cx.p_out = ec(tc.tile_pool(name="outp", bufs=2))
        cx.p_sml = ec(tc.tile_pool(name="sml", bufs=2))
        cx.p_mv = ec(tc.tile_pool(name="mv", bufs=10))
        cx.ps_wide = ec(tc.tile_pool(name="wide", bufs=2, space="PSUM"))
        cx.ps_half = ec(tc.tile_pool(name="half", bufs=2, space="PSUM"))
        cx.ps_pv = ec(tc.tile_pool(name="pv", bufs=2, space="PSUM"))
        _setup_consts(nc, cx, cst, flags)

        def body():
            for b in range(BL):
                rb = b * NQ
                QT = _p1_transpose(nc, cx, rb, dQ, "QT")
                KT = _p1_transpose(nc, cx, rb, dK, "KT")
                qT, kT, vT = _p2_proj(nc, cx, QT, KT)
                XT = cx.p_xt.tile([128, 4, NQ], BF16, tag="XT")
                SQ = cx.p_xt.tile([128, 4, NQ], BF16, tag="SQ")
                for hp in range(4):
                    for qc in range(2):
                        _p3_attn_unit(nc, cx, qT, kT, vT, XT, SQ, hp, qc)
                XnT = _p4_ln0(nc, cx, XT, SQ, ln0_aff)
                _p5_out(nc, cx, XnT, dOut, rb, ln1_aff)

        if repeat == 1:
            body()
        else:
            with tc.For_i(0, repeat, 1):
                body()

    nc.compile()
    return nc


def _consts(Wq, Wk, Wv, Wo, flags, bq, bk, bv, bo, g0, b0, g1, b1):
    (bq_nz, bk_nz, bv_nz, bo_nz, ln0_aff, ln1_aff) = flags
    c = {
        "Wqb": np.ascontiguousarray(np.asarray(Wq).astype(NBF)),
        "Wkb": np.ascontiguousarray(np.asarray(Wk).astype(NBF)),
        "Wvb": np.ascontiguousarray(np.asarray(Wv).astype(NBF)),
        "Wob": np.ascontiguousarray(np.asarray(Wo).astype(NBF)),
        "I512b": np.eye(D, dtype=NBF),
        "I128b": np.eye(128, dtype=NBF),
        "onesc": np.ones((128, 1), NBF),
        "onesr": np.ones((1, 128), NBF),
    }
    mk2 = np.zeros((2, 128), NBF)
    mk2[0, :64] = 1
    mk2[1, 64:] = 1
    c["mk2"] = mk2
    if bq_nz: c["bq4"] = np.ascontiguousarray(np.asarray(bq).reshape(4, 128).T.astype(np.float32))
    if bk_nz: c["bk4"] = np.ascontiguousarray(np.asarray(bk).reshape(4, 128).T.astype(np.float32))
    if bv_nz: c["bvb"] = np.ascontiguousarray(np.broadcast_to(np.asarray(bv, np.float32), (128, D)))
    if bo_nz: c["bob"] = np.ascontiguousarray(np.broadcast_to(np.asarray(bo, np.float32), (128, D)))
    if ln0_aff:
        c["g04"] = np.ascontiguousarray(np.asarray(g0).reshape(4, 128).T.astype(np.float32))
        c["b04"] = np.ascontiguousarray(np.asarray(b0).reshape(4, 128).T.astype(np.float32))
    if ln1_aff:
        c["g1b"] = np.ascontiguousarray(np.broadcast_to(np.asarray(g1, np.float32), (128, D)))
        c["b1b"] = np.ascontiguousarray(np.broadcast_to(np.asarray(b1, np.float32), (128, D)))
    return c


def make_in_maps(Q, K, Wq, bq, Wk, bk, Wv, bv, Wo, bo, g0, b0, g1, b1, flags):
    consts = _consts(Wq, Wk, Wv, Wo, flags, bq, bk, bv, bo, g0, b0, g1, b1)
    in_maps = []
    for ci in range(N_CORES):
        m = dict(consts)
        m["Qs"] = np.ascontiguousarray(
            np.asarray(Q)[ci * BL:(ci + 1) * BL].reshape(BL * NQ, D).astype(np.float32))
        m["Ks"] = np.ascontiguousarray(
            np.asarray(K)[ci * BL:(ci + 1) * BL].reshape(BL * NK, D).astype(np.float32))
        in_maps.append(m)
    return in_maps


def get_flags(bq, bk, bv, bo, g0, b0, g1, b1):
    return (bool(np.any(np.asarray(bq))), bool(np.any(np.asarray(bk))),
            bool(np.any(np.asarray(bv))), bool(np.any(np.asarray(bo))),
            bool(np.any(np.asarray(g0) != 1) or np.any(np.asarray(b0))),
            bool(np.any(np.asarray(g1) != 1) or np.any(np.asarray(b1))))


def get_program(flags, repeat=1):
    key = (flags, repeat)
    if key not in _cache:
        _cache[key] = _build(flags, repeat)
    return _cache[key]


def kernel(Q, K, Wq, bq, Wk, bk, Wv, bv, Wo, bo, g0, b0, g1, b1):
    flags = get_flags(bq, bk, bv, bo, g0, b0, g1, b1)
    nc = get_program(flags, repeat=1)
    in_maps = make_in_maps(Q, K, Wq, bq, Wk, bk, Wv, bv, Wo, bo, g0, b0, g1, b1, flags)
    res = run_bass_kernel_spmd(nc, in_maps, list(range(N_CORES)))
    out = np.empty((B, NQ, D), np.float32)
    for ci in range(N_CORES):
        out[ci * BL:(ci + 1) * BL] = res.results[ci]["OUT"].reshape(BL, NQ, D)
    return out



# revision 37
# speedup vs baseline: 3080.5034x; 3080.5034x over previous
"""Trainium2 Bass kernel for MAB (multihead attention block) — nn_MAB_48412871360901.

Data-parallel over batch: 16 batches -> 8 NeuronCores, 2 batches/core.
Per core, per batch (all matmuls bf16 with fp32 PSUM accumulation):
  P1  Q,K loaded natural (batched 4-chunk DMAs), cast bf16, transposed to
      QT/KT [dv, nq] layout via DMA xbar transpose (no PE involvement)
  P2  projections qT = Wq^T QT, kT = Wk^T KT (transposed layout), v = K Wv (natural)
  P3  per (head-pair, q-chunk): S^T = k^T.T q^T (row-packed 2 heads),
      exp on ACT (scale 1/sqrt(512) fused), softmax denominators Z via
      ones-matmuls, PV U^T = v^T expS^T (col-packed 2 heads),
      X^T = U^T * (1/Z)bcast + qT   (residual uses post-projection q)
  P4  LN0 in transposed layout: stats via ones-matmuls over partitions,
      rstd = exp(-0.5 ln(var+eps)) on ACT, normalize with PE-broadcast tiles
  P5  Xn natural obtained via DMA xbar transpose of XnT; M = Xn Wo (natural
      out from XnT lhsT); relu; residual; LN1 natural (bn_stats); batched
      DMA out fp32.

All scalar-engine activations are Exp/Ln, forced into the single
natural_log_exp_and_others table set to avoid ACT table-load thrash.
"""

import sys
import functools
from contextlib import ExitStack
import numpy as np
import ml_dtypes

for _p in ("/opt/trn_rl_repo", "/root/.axon_site/_ro/trn_rl_repo"):
    if _p not in sys.path:
        sys.path.insert(0, _p)

import concourse.bacc as bacc
import concourse.mybir as mybir
import concourse.tile as tile
from concourse.bass_utils import run_bass_kernel_spmd
from concourse.hw_specs import get_activation_tables as _orig_gat

BF16 = mybir.dt.bfloat16
F32 = mybir.dt.float32
NBF = ml_dtypes.bfloat16
AF = mybir.ActivationFunctionType
OP = mybir.AluOpType

B, NQ, NK = 16, 1024, 1024
D = 512
H = 8
N_CORES = 8
BL = B // N_CORES          # batches per core
EPS = 1e-5
SCALE = 1.0 / np.sqrt(512.0)

_ONE_SET = "natural_log_exp_and_others"


@functools.cache
def _gat_one_set(arch):
    """Empty out every activation-table set except the one containing both
    Exp and Ln, so bacc's table-load pass emits a single LoadActFuncSet
    instead of thrashing between exp_and_others and natural_log.
    Set indices (act_func_set_id) are preserved."""
    tabs = _orig_gat(arch)
    return {name: (fns if name == _ONE_SET else frozenset())
            for name, fns in tabs.items()}


bacc.get_activation_tables = _gat_one_set

_cache = {}


class _Ctx:
    pass


def _setup_consts(nc, cx, cst, flags):
    (bq_nz, bk_nz, bv_nz, bo_nz, ln0_aff, ln1_aff) = flags

    def din(name, shape, dt=BF16):
        return nc.dram_tensor(name, list(shape), dt, kind="ExternalInput").ap()

    def ldc(name, dshape, shape, rearr=None):
        d = din(name, dshape)
        t = cst.tile(list(shape), BF16, tag=name)
        nc.sync.dma_start(out=t, in_=d if rearr is None else d.rearrange(rearr, p=128))
        return t

    def ldf(name, shape):
        d = din(name, shape, F32)
        t = cst.tile(list(shape), F32, tag=name)
        nc.sync.dma_start(out=t, in_=d)
        return t

    cx.w_q = ldc("Wqb", (D, D), (128, 4, D), "(kt p) c -> p kt c")
    cx.w_k = ldc("Wkb", (D, D), (128, 4, D), "(kt p) c -> p kt c")
    cx.w_v = ldc("Wvb", (D, D), (128, 4, D), "(kt p) c -> p kt c")
    cx.w_o = ldc("Wob", (D, D), (128, 4, D), "(kt p) c -> p kt c")
    cx.onesc = ldc("onesc", (128, 1), (128, 1))
    cx.ones64 = ldc("ones64", (128, 64), (128, 64))
    cx.onesr = ldc("onesr", (1, 128), (1, 128))
    cx.epsP = cst.tile([128, 1], F32, tag="epsP"); nc.vector.memset(cx.epsP, EPS)
    cx.eps1 = cst.tile([1, 1], F32, tag="eps1"); nc.vector.memset(cx.eps1, EPS)
    cx.bq4 = ldf("bq4", (128, 4)) if bq_nz else None
    cx.bk4 = ldf("bk4", (128, 4)) if bk_nz else None
    cx.bvb = ldf("bvb", (128, D)) if bv_nz else None
    cx.bob = ldf("bob", (128, D)) if bo_nz else None
    cx.g04 = ldf("g04", (128, 4)) if ln0_aff else None
    cx.b04 = ldf("b04", (128, 4)) if ln0_aff else None
    cx.g1b = ldf("g1b", (128, D)) if ln1_aff else None
    cx.b1b = ldf("b1b", (128, D)) if ln1_aff else None


def _p1_transpose(nc, cx, rb, src_dram, tag):
    """DMA-xbar-transpose bf16 input (host-cast) straight from DRAM into the
    [dv, nq] layout.

    dma_start_transpose maps transposed row r of a [128, 512] input to
    out[p, kt, c] with r = kt*128 + p — the same (kt p) layout the weight
    tiles use, so the projection matmuls consume dst directly."""
    dst = cx.p_qkt.tile([128, 4, NQ], BF16, tag=tag)
    for i in range(8):
        nc.sync.dma_start_transpose(
            out=dst[:, :, 128 * i:128 * (i + 1)],
            in_=src_dram[rb + 128 * i: rb + 128 * (i + 1), :])
    return dst


def _p2_alloc(cx):
    qT = cx.p_proj.tile([128, 4, NQ], BF16, tag="qT")
    kT = cx.p_proj.tile([128, 4, NQ], BF16, tag="kT")
    vT = cx.p_proj.tile([128, 8, D], BF16, tag="vT")
    return qT, kT, vT


def _p2_chains(nc, cx, proj, QT, KT):
    """Return the 24 projection chains (4 PE matmuls + evac each) as
    closures, so the body can interleave their EMISSION into other phases —
    each engine's instruction stream executes in program order, so filler
    work must be woven in at emission time."""
    qT, kT, vT = proj
    chains = []

    def qk_chain(dst, w, srcT, bias, dvt, qc):
        def emit():
            pp = cx.ps_half.tile([128, D], F32, tag="h")
            for kt in range(4):
                nc.tensor.matmul(
                    pp, lhsT=w[:, kt, 128 * dvt:128 * (dvt + 1)],
                    rhs=srcT[:, kt, 512 * qc:512 * (qc + 1)],
                    start=(kt == 0), stop=(kt == 3))
            o = dst[:, dvt, 512 * qc:512 * (qc + 1)]
            if bias is not None:
                nc.vector.tensor_scalar_add(out=o, in0=pp, scalar1=bias[:, dvt:dvt + 1])
            else:
                nc.vector.tensor_copy(out=o, in_=pp)
        return emit

    def v_chain(nkt):
        def emit():
            pp = cx.ps_half.tile([128, D], F32, tag="h")
            for kt in range(4):
                nc.tensor.matmul(pp, lhsT=KT[:, kt, 128 * nkt:128 * (nkt + 1)],
                                 rhs=cx.w_v[:, kt, :], start=(kt == 0), stop=(kt == 3))
            if cx.bvb is not None:
                nc.vector.scalar_tensor_tensor(out=vT[:, nkt, :], in0=pp, scalar=0.0,
                                               in1=cx.bvb, op0=OP.add, op1=OP.add)
            else:
                nc.vector.tensor_copy(out=vT[:, nkt, :], in_=pp)
        return emit

    for dvt in range(4):
        for qc in range(2):
            chains.append(qk_chain(qT, cx.w_q, QT, cx.bq4, dvt, qc))
    for dvt in range(4):
        for qc in range(2):
            chains.append(qk_chain(kT, cx.w_k, KT, cx.bk4, dvt, qc))
    for nkt in range(8):
        chains.append(v_chain(nkt))
    return chains


def _p3_attn_unit(nc, cx, qT, kT, vT, XT, SQ, hp, qc, filler=None, fill_at=()):
    ps_u = cx.ps_pv.tile([128, D], F32, tag="u")
    ps_z = cx.ps_z.tile([128, D], F32, tag="z")
    for kt in range(8):
        ps_s = cx.ps_wide.tile([128, 2 * D], F32, tag="w")
        nc.tensor.matmul(
            ps_s[:, 0:D],
            lhsT=kT[0:64, hp, 128 * kt:128 * (kt + 1)],
            rhs=qT[0:64, hp, 512 * qc:512 * (qc + 1)],
            start=True, stop=True, tile_position=(0, 0))
        nc.tensor.matmul(
            ps_s[:, D:2 * D],
            lhsT=kT[64:128, hp, 128 * kt:128 * (kt + 1)],
            rhs=qT[64:128, hp, 512 * qc:512 * (qc + 1)],
            start=True, stop=True, tile_position=(64, 0))
        ex = cx.p_ex.tile([128, 2 * D], BF16)
        nc.scalar.activation(out=ex, in_=ps_s, func=AF.Exp, scale=SCALE)
        nc.tensor.matmul(ps_z[0:64, :], lhsT=cx.ones64, rhs=ex[:, 0:D],
                         start=(kt == 0), stop=(kt == 7), tile_position=(0, 0))
        nc.tensor.matmul(ps_z[64:128, :], lhsT=cx.ones64, rhs=ex[:, D:2 * D],
                         start=(kt == 0), stop=(kt == 7), tile_position=(0, 64))
        nc.tensor.matmul(ps_u[0:64, :],
                         lhsT=vT[:, kt, 128 * hp:128 * hp + 64],
                         rhs=ex[:, 0:D],
                         start=(kt == 0), stop=(kt == 7), tile_position=(0, 0))
        nc.tensor.matmul(ps_u[64:128, :],
                         lhsT=vT[:, kt, 128 * hp + 64:128 * (hp + 1)],
                         rhs=ex[:, D:2 * D],
                         start=(kt == 0), stop=(kt == 7), tile_position=(0, 64))
        if filler and kt in fill_at:
            filler.popleft()()
    # ps_z rows 0:64 all hold Z(head 2hp), rows 64:128 all Z(head 2hp+1)
    # (ones64 lhsT, same N-bound matmul cost as an M=1 reduction), so the
    # reciprocal is already partition-broadcast.
    rz = cx.p_rzb.tile([128, D], BF16, tag="rzb")
    with nc.allow_low_precision(reason="softmax 1/Z in bf16 is consistent with bf16 probs"):
        nc.vector.reciprocal(out=rz, in_=ps_z)
    t1 = cx.p_t1.tile([128, D], BF16, tag="t")
    nc.vector.tensor_tensor(out=t1, in0=ps_u, in1=rz, op=OP.mult)
    xs = XT[:, hp, 512 * qc:512 * (qc + 1)]
    nc.vector.tensor_tensor(out=xs, in0=t1,
                            in1=qT[:, hp, 512 * qc:512 * (qc + 1)], op=OP.add)
    nc.vector.tensor_tensor(out=SQ[:, hp, 512 * qc:512 * (qc + 1)],
                            in0=xs, in1=xs, op=OP.mult)


def _p4_ln0(nc, cx, XT, SQ, ln0_aff):
    XnT = cx.p_xnt.tile([128, 4, NQ], BF16)
    for qc in range(2):
        ps_st = cx.ps_half.tile([128, D], F32, tag="h")
        for dvt in range(4):
            nc.tensor.matmul(ps_st[0:1, :], lhsT=cx.onesc,
                             rhs=XT[:, dvt, 512 * qc:512 * (qc + 1)],
                             start=(dvt == 0), stop=(dvt == 3), tile_position=(0, 0))
            nc.tensor.matmul(ps_st[32:33, :], lhsT=cx.onesc,
                             rhs=SQ[:, dvt, 512 * qc:512 * (qc + 1)],
                             start=(dvt == 0), stop=(dvt == 3), tile_position=(0, 32))
        mu = cx.p_sml.tile([1, D], F32, tag="mu")
        nc.vector.tensor_scalar_mul(out=mu, in0=ps_st[0:1, :], scalar1=1.0 / D)
        mu2 = cx.p_sml.tile([1, D], F32, tag="mu2")
        nc.vector.tensor_tensor(out=mu2, in0=mu, in1=mu, op=OP.mult)
        var = cx.p_sml.tile([1, D], F32, tag="var")
        nc.vector.scalar_tensor_tensor(out=var, in0=ps_st[32:33, :],
                                       scalar=1.0 / D, in1=mu2,
                                       op0=OP.mult, op1=OP.subtract)
        lnv = cx.p_sml.tile([1, D], F32, tag="lnv")
        nc.scalar.activation(out=lnv, in_=var, func=AF.Ln, bias=cx.eps1, scale=1.0)
        rstd = cx.p_sml.tile([1, D], BF16, tag="rstd")
        nc.scalar.activation(out=rstd, in_=lnv, func=AF.Exp, scale=-0.5)
        nmr = cx.p_sml.tile([1, D], BF16, tag="nmr")
        nc.vector.scalar_tensor_tensor(out=nmr, in0=mu, scalar=-1.0, in1=rstd,
                                       op0=OP.mult, op1=OP.mult)
        rstdb = cx.p_rzb.tile([128, D], BF16, tag="rstdb")
        nc.gpsimd.partition_broadcast(rstdb, rstd)
        nmrb = cx.p_rzb.tile([128, D], BF16, tag="nmrb")
        nc.gpsimd.partition_broadcast(nmrb, nmr)
        for dvt in range(4):
            t2 = cx.p_t1.tile([128, D], BF16, tag="t")
            nc.vector.tensor_tensor(out=t2, in0=XT[:, dvt, 512 * qc:512 * (qc + 1)],
                                    in1=rstdb, op=OP.mult)
            xn = XnT[:, dvt, 512 * qc:512 * (qc + 1)]
            nc.vector.tensor_tensor(out=xn, in0=t2, in1=nmrb, op=OP.add)
            if ln0_aff:
                nc.vector.tensor_scalar(out=xn, in0=xn,
                                        scalar1=cx.g04[:, dvt:dvt + 1],
                                        scalar2=cx.b04[:, dvt:dvt + 1],
                                        op0=OP.mult, op1=OP.add)
    return XnT


class _P5State:
    pass


def _p5_start(nc, cx, XnT):
    # Xn natural [nq, dv] via DMA xbar transpose: out[p, nqt, dvt, c] with
    # nq = nqt*128 + p, dv = dvt*128 + c.  ACT-issued: separate HWDGE queue
    # from the P1 input transposes on SP.
    st = _P5State()
    st.XnT = XnT
    st.Xn = cx.p_xn.tile([128, 8, 4, 128], BF16, tag="Xn")
    for dvt in range(4):
        nc.scalar.dma_start_transpose(out=st.Xn[:, :, dvt, :], in_=XnT[:, dvt, :])
    st.xpre_l, st.mv_l = [], []
    st.vars8 = cx.p_sml.tile([128, 8], F32, tag="vars8")
    return st


def _p5_chunk(nc, cx, st, nqt):
    XnT, Xn = st.XnT, st.Xn
    ps_m = cx.ps_half.tile([128, D], F32, tag="h")
    for dvt in range(4):
        nc.tensor.matmul(ps_m, lhsT=XnT[:, dvt, 128 * nqt:128 * (nqt + 1)],
                         rhs=cx.w_o[:, dvt, :],
                         start=(dvt == 0), stop=(dvt == 3))
    rl = cx.p_t1.tile([128, D], BF16, tag="t")
    if cx.bob is not None:
        tb = cx.p_t1.tile([128, D], BF16, tag="t")
        nc.vector.tensor_tensor(out=tb, in0=cx.bob, in1=ps_m, op=OP.add)
        nc.vector.tensor_scalar_max(out=rl, in0=tb, scalar1=0.0)
    else:
        nc.vector.tensor_scalar_max(out=rl, in0=ps_m, scalar1=0.0)
    xpre = cx.p_xp.tile([128, D], F32)
    nc.vector.tensor_tensor(out=xpre, in0=rl,
                            in1=Xn[:, nqt, :, :].rearrange("p a b -> p (a b)"),
                            op=OP.add)
    bst = cx.p_mv.tile([128, 6], F32, tag="bst")
    nc.vector.bn_stats(out=bst, in_=xpre)
    mv = cx.p_mv.tile([128, 2], F32, tag="mv")
    nc.vector.bn_aggr(out=mv, in_=bst)
    nc.vector.tensor_copy(out=st.vars8[:, nqt:nqt + 1], in_=mv[:, 1:2])
    st.xpre_l.append(xpre); st.mv_l.append(mv)


def _p5_finish(nc, cx, st, dOut, rb, ln1_aff):
    lnv8 = cx.p_sml.tile([128, 8], F32, tag="lnv8")
    nc.scalar.activation(out=lnv8, in_=st.vars8, func=AF.Ln, bias=cx.epsP, scale=1.0)
    rstd8 = cx.p_sml.tile([128, 8], F32, tag="rstd8")
    nc.scalar.activation(out=rstd8, in_=lnv8, func=AF.Exp, scale=-0.5)
    for g in range(2):
        out4 = cx.p_out.tile([128, 4, D], F32)
        for c in range(4):
            nqt = 4 * g + c
            ot = out4[:, c, :]
            nc.vector.tensor_scalar(out=ot, in0=st.xpre_l[nqt],
                                    scalar1=st.mv_l[nqt][:, 0:1],
                                    scalar2=rstd8[:, nqt:nqt + 1],
                                    op0=OP.subtract, op1=OP.mult)
            if ln1_aff:
                nc.vector.tensor_tensor(out=ot, in0=ot, in1=cx.g1b, op=OP.mult)
                nc.vector.tensor_tensor(out=ot, in0=ot, in1=cx.b1b, op=OP.add)
        nc.scalar.dma_start(
            out=dOut[rb + 512 * g: rb + 512 * (g + 1), :].rearrange(
                "(c p) d -> p c d", p=128),
            in_=out4)


def _build(flags, repeat=1):
    (bq_nz, bk_nz, bv_nz, bo_nz, ln0_aff, ln1_aff) = flags
    nc = bacc.Bacc("TRN2", target_bir_lowering=False, debug=False,
                   num_devices=N_CORES)

    dQ = nc.dram_tensor("Qs", [BL * NQ, D], BF16, kind="ExternalInput").ap()
    dK = nc.dram_tensor("Ks", [BL * NK, D], BF16, kind="ExternalInput").ap()
    dOut = nc.dram_tensor("OUT", [BL * NQ, D], F32, kind="ExternalOutput").ap()

    cx = _Ctx()
    with ExitStack() as es:
        tc = es.enter_context(tile.TileContext(nc))
        ec = es.enter_context
        cst = ec(tc.tile_pool(name="cst", bufs=1))
        cx.p_qkt = ec(tc.tile_pool(name="qkt", bufs=1))
        cx.p_proj = ec(tc.tile_pool(name="proj", bufs=2))
        cx.p_xt = ec(tc.tile_pool(name="xt", bufs=2))
        cx.p_xnt = ec(tc.tile_pool(name="xnt", bufs=2))
        cx.p_xn = ec(tc.tile_pool(name="xn", bufs=1))
        cx.p_ex = ec(tc.tile_pool(name="ex", bufs=4))
        cx.p_rzb = ec(tc.tile_pool(name="rzb", bufs=2))
        cx.p_t1 = ec(tc.tile_pool(name="t1", bufs=4))
        cx.p_xp = ec(tc.tile_pool(name="xp", bufs=9))
        cx.p_out = ec(tc.tile_pool(name="outp", bufs=2))
        cx.p_sml = ec(tc.tile_pool(name="sml", bufs=1))
        cx.p_mv = ec(tc.tile_pool(name="mv", bufs=10))
        cx.ps_wide = ec(tc.tile_pool(name="wide", bufs=2, space="PSUM"))
        cx.ps_half = ec(tc.tile_pool(name="half", bufs=2, space="PSUM"))
        cx.ps_pv = ec(tc.tile_pool(name="pv", bufs=1, space="PSUM"))
        cx.ps_z = ec(tc.tile_pool(name="z", bufs=1, space="PSUM"))
        _setup_consts(nc, cx, cst, flags)

        def body():
            from collections import deque
            units = [(hp, qc) for hp in range(4) for qc in range(2)]
            # Batch 0 inputs, then batch 1 inputs (all DMA-only, queue early)
            QT0 = _p1_transpose(nc, cx, 0, dQ, "QT")
            KT0 = _p1_transpose(nc, cx, 0, dK, "KT")
            proj0 = _p2_alloc(cx)
            for c in _p2_chains(nc, cx, proj0, QT0, KT0):
                c()
            QT1 = _p1_transpose(nc, cx, NQ, dQ, "QT")
            KT1 = _p1_transpose(nc, cx, NQ, dK, "KT")
            qT0, kT0, vT0 = proj0
            # P3 batch 0, with batch-1 projection chains woven in as PE
            # filler (P3 is ACT-bound; each engine stream runs in program
            # order, so filler must be emitted inline).
            proj1 = _p2_alloc(cx)
            fill = deque(_p2_chains(nc, cx, proj1, QT1, KT1))
            XT0 = cx.p_xt.tile([128, 4, NQ], BF16, tag="XT")
            SQ0 = cx.p_xt.tile([128, 4, NQ], BF16, tag="SQ")
            for hp, qc in units:
                _p3_attn_unit(nc, cx, qT0, kT0, vT0, XT0, SQ0, hp, qc,
                              filler=fill, fill_at=(2, 4, 6))
            qT1, kT1, vT1 = proj1
            # P3 batch 1, with batch-0 epilogue (P4 + P5) woven in.
            XT1 = cx.p_xt.tile([128, 4, NQ], BF16, tag="XT")
            SQ1 = cx.p_xt.tile([128, 4, NQ], BF16, tag="SQ")
            st0_box = []

            def f_p4():
                st0_box.append(_p5_start(nc, cx, _p4_ln0(nc, cx, XT0, SQ0, ln0_aff)))

            def f_chunk(nqt):
                return lambda: _p5_chunk(nc, cx, st0_box[0], nqt)

            def f_finish():
                _p5_finish(nc, cx, st0_box[0], dOut, 0, ln1_aff)

            fill = deque([f_p4] + [f_chunk(i) for i in range(8)] + [f_finish])
            fills = {0: (4,), 1: (2, 5), 2: (2, 5), 3: (2, 5), 4: (2, 5), 5: (2, 5)}
            for u, (hp, qc) in enumerate(units):
                _p3_attn_unit(nc, cx, qT1, kT1, vT1, XT1, SQ1, hp, qc,
                              filler=fill, fill_at=fills.get(u, ()))
            while fill:
                fill.popleft()()
            # Batch 1 epilogue (tail)
            XnT1 = _p4_ln0(nc, cx, XT1, SQ1, ln0_aff)
            st1 = _p5_start(nc, cx, XnT1)
            for nqt in range(8):
                _p5_chunk(nc, cx, st1, nqt)
            _p5_finish(nc, cx, st1, dOut, NQ, ln1_aff)

        if repeat == 1:
            body()
        else:
            with tc.For_i(0, repeat, 1):
                body()

    nc.compile()
    return nc


def _consts(Wq, Wk, Wv, Wo, flags, bq, bk, bv, bo, g0, b0, g1, b1):
    (bq_nz, bk_nz, bv_nz, bo_nz, ln0_aff, ln1_aff) = flags
    c = {
        "Wqb": np.ascontiguousarray(np.asarray(Wq).astype(NBF)),
        "Wkb": np.ascontiguousarray(np.asarray(Wk).astype(NBF)),
        "Wvb": np.ascontiguousarray(np.asarray(Wv).astype(NBF)),
        "Wob": np.ascontiguousarray(np.asarray(Wo).astype(NBF)),
        "onesc": np.ones((128, 1), NBF),
        "ones64": np.ones((128, 64), NBF),
        "onesr": np.ones((1, 128), NBF),
    }
    if bq_nz: c["bq4"] = np.ascontiguousarray(np.asarray(bq).reshape(4, 128).T.astype(np.float32))
    if bk_nz: c["bk4"] = np.ascontiguousarray(np.asarray(bk).reshape(4, 128).T.astype(np.float32))
    if bv_nz: c["bvb"] = np.ascontiguousarray(np.broadcast_to(np.asarray(bv, np.float32), (128, D)))
    if bo_nz: c["bob"] = np.ascontiguousarray(np.broadcast_to(np.asarray(bo, np.float32), (128, D)))
    if ln0_aff:
        c["g04"] = np.ascontiguousarray(np.asarray(g0).reshape(4, 128).T.astype(np.float32))
        c["b04"] = np.ascontiguousarray(np.asarray(b0).reshape(4, 128).T.astype(np.float32))
    if ln1_aff:
        c["g1b"] = np.ascontiguousarray(np.broadcast_to(np.asarray(g1, np.float32), (128, D)))
        c["b1b"] = np.ascontiguousarray(np.broadcast_to(np.asarray(b1, np.float32), (128, D)))
    return c


def make_in_maps(Q, K, Wq, bq, Wk, bk, Wv, bv, Wo, bo, g0, b0, g1, b1, flags):
    consts = _consts(Wq, Wk, Wv, Wo, flags, bq, bk, bv, bo, g0, b0, g1, b1)
    in_maps = []
    for ci in range(N_CORES):
        m = dict(consts)
        m["Qs"] = np.ascontiguousarray(
            np.asarray(Q)[ci * BL:(ci + 1) * BL].reshape(BL * NQ, D).astype(NBF))
        m["Ks"] = np.ascontiguousarray(
            np.asarray(K)[ci * BL:(ci + 1) * BL].reshape(BL * NK, D).astype(NBF))
        in_maps.append(m)
    return in_maps


def get_flags(bq, bk, bv, bo, g0, b0, g1, b1):
    return (bool(np.any(np.asarray(bq))), bool(np.any(np.asarray(bk))),
            bool(np.any(np.asarray(bv))), bool(np.any(np.asarray(bo))),
            bool(np.any(np.asarray(g0) != 1) or np.any(np.asarray(b0))),
            bool(np.any(np.asarray(g1) != 1) or np.any(np.asarray(b1))))


def get_program(flags, repeat=1):
    key = (flags, repeat)
    if key not in _cache:
        _cache[key] = _build(flags, repeat)
    return _cache[key]


def kernel(Q, K, Wq, bq, Wk, bk, Wv, bv, Wo, bo, g0, b0, g1, b1):
    flags = get_flags(bq, bk, bv, bo, g0, b0, g1, b1)
    nc = get_program(flags, repeat=1)
    in_maps = make_in_maps(Q, K, Wq, bq, Wk, bk, Wv, bv, Wo, bo, g0, b0, g1, b1, flags)
    res = run_bass_kernel_spmd(nc, in_maps, list(range(N_CORES)))
    out = np.empty((B, NQ, D), np.float32)
    for ci in range(N_CORES):
        out[ci * BL:(ci + 1) * BL] = res.results[ci]["OUT"].reshape(BL, NQ, D)
    return out


# revision 48
# speedup vs baseline: 3467.8125x; 1.1257x over previous
"""Trainium2 Bass kernel for MAB (multihead attention block) — nn_MAB_48412871360901.

Data-parallel over batch: 16 batches -> 8 NeuronCores, 2 batches/core.
Per core, per batch (all matmuls bf16 with fp32 PSUM accumulation):
  P1  Q,K loaded natural (batched 4-chunk DMAs), cast bf16, transposed to
      QT/KT [dv, nq] layout via DMA xbar transpose (no PE involvement)
  P2  projections qT = Wq^T QT, kT = Wk^T KT (transposed layout), v = K Wv (natural)
  P3  per (head-pair, q-chunk): S^T = k^T.T q^T (row-packed 2 heads),
      exp on ACT (scale 1/sqrt(512) fused), softmax denominators Z via
      ones-matmuls, PV U^T = v^T expS^T (col-packed 2 heads),
      X^T = U^T * (1/Z)bcast + qT   (residual uses post-projection q)
  P4  LN0 in transposed layout: stats via ones-matmuls over partitions,
      rstd = exp(-0.5 ln(var+eps)) on ACT, normalize with PE-broadcast tiles
  P5  Xn natural obtained via DMA xbar transpose of XnT; M = Xn Wo (natural
      out from XnT lhsT); relu; residual; LN1 natural (bn_stats); batched
      DMA out fp32.

All scalar-engine activations are Exp/Ln, forced into the single
natural_log_exp_and_others table set to avoid ACT table-load thrash.
"""

import sys
import functools
from contextlib import ExitStack
import numpy as np
import ml_dtypes

for _p in ("/opt/trn_rl_repo", "/root/.axon_site/_ro/trn_rl_repo"):
    if _p not in sys.path:
        sys.path.insert(0, _p)

import concourse.bacc as bacc
import concourse.mybir as mybir
import concourse.tile as tile
from concourse.bass_utils import run_bass_kernel_spmd
from concourse.hw_specs import get_activation_tables as _orig_gat

BF16 = mybir.dt.bfloat16
F32 = mybir.dt.float32
NBF = ml_dtypes.bfloat16
AF = mybir.ActivationFunctionType
OP = mybir.AluOpType

B, NQ, NK = 16, 1024, 1024
D = 512
H = 8
N_CORES = 8
BL = B // N_CORES          # batches per core
EPS = 1e-5
SCALE = 1.0 / np.sqrt(512.0)

_ONE_SET = "natural_log_exp_and_others"


@functools.cache
def _gat_one_set(arch):
    """Empty out every activation-table set except the one containing both
    Exp and Ln, so bacc's table-load pass emits a single LoadActFuncSet
    instead of thrashing between exp_and_others and natural_log.
    Set indices (act_func_set_id) are preserved."""
    tabs = _orig_gat(arch)
    return {name: (fns if name == _ONE_SET else frozenset())
            for name, fns in tabs.items()}


bacc.get_activation_tables = _gat_one_set

_cache = {}


class _Ctx:
    pass


def _setup_consts(nc, cx, cst, flags):
    (bq_nz, bk_nz, bv_nz, bo_nz, ln0_aff, ln1_aff) = flags

    def din(name, shape, dt=BF16):
        return nc.dram_tensor(name, list(shape), dt, kind="ExternalInput").ap()

    def ldc(name, dshape, shape, rearr=None):
        d = din(name, dshape)
        t = cst.tile(list(shape), BF16, tag=name)
        nc.sync.dma_start(out=t, in_=d if rearr is None else d.rearrange(rearr, p=128))
        return t

    def ldf(name, shape):
        d = din(name, shape, F32)
        t = cst.tile(list(shape), F32, tag=name)
        nc.sync.dma_start(out=t, in_=d)
        return t

    cx.w_q = ldc("Wqb", (D, D), (128, 4, D), "(kt p) c -> p kt c")
    cx.w_k = ldc("Wkb", (D, D), (128, 4, D), "(kt p) c -> p kt c")
    cx.w_v = ldc("Wvb", (D, D), (128, 4, D), "(kt p) c -> p kt c")
    cx.w_o = ldc("Wob", (D, D), (128, 4, D), "(kt p) c -> p kt c")
    cx.onesc = ldc("onesc", (128, 1), (128, 1))
    cx.ones64 = ldc("ones64", (128, 64), (128, 64))
    cx.epsP = cst.tile([128, 1], F32, tag="epsP"); nc.vector.memset(cx.epsP, EPS)
    cx.eps1 = cst.tile([1, 1], F32, tag="eps1"); nc.vector.memset(cx.eps1, EPS)
    cx.bq4 = ldf("bq4", (128, 4)) if bq_nz else None
    cx.bk4 = ldf("bk4", (128, 4)) if bk_nz else None
    cx.bvb = ldf("bvb", (128, D)) if bv_nz else None
    cx.bob = ldf("bob", (128, D)) if bo_nz else None
    cx.g04 = ldf("g04", (128, 4)) if ln0_aff else None
    cx.b04 = ldf("b04", (128, 4)) if ln0_aff else None
    cx.g1b = ldf("g1b", (128, D)) if ln1_aff else None
    cx.b1b = ldf("b1b", (128, D)) if ln1_aff else None


def _p1_transpose(nc, cx, rb, src_dram, tag):
    """DMA-xbar-transpose bf16 input (host-cast) straight from DRAM into the
    [dv, nq] layout.

    dma_start_transpose maps transposed row r of a [128, 512] input to
    out[p, kt, c] with r = kt*128 + p — the same (kt p) layout the weight
    tiles use, so the projection matmuls consume dst directly."""
    dst = cx.p_qkt.tile([128, 4, NQ], BF16, tag=tag)
    for i in range(8):
        nc.sync.dma_start_transpose(
            out=dst[:, :, 128 * i:128 * (i + 1)],
            in_=src_dram[rb + 128 * i: rb + 128 * (i + 1), :])
    return dst


def _p2_alloc(cx):
    qT = cx.p_proj.tile([128, 4, NQ], BF16, tag="qT")
    kT = cx.p_proj.tile([128, 4, NQ], BF16, tag="kT")
    vT = cx.p_proj.tile([128, 8, D], BF16, tag="vT")
    return qT, kT, vT


def _p2_chains(nc, cx, proj, QT, KT):
    """Return the 24 projection chains (4 PE matmuls + evac each) as
    closures, so the body can interleave their EMISSION into other phases —
    each engine's instruction stream executes in program order, so filler
    work must be woven in at emission time."""
    qT, kT, vT = proj
    chains = []

    def qk_chain(dst, w, srcT, bias, dvt, qc):
        def emit():
            pp = cx.ps_half.tile([128, D], F32, tag="h")
            for kt in range(4):
                nc.tensor.matmul(
                    pp, lhsT=w[:, kt, 128 * dvt:128 * (dvt + 1)],
                    rhs=srcT[:, kt, 512 * qc:512 * (qc + 1)],
                    start=(kt == 0), stop=(kt == 3))
            o = dst[:, dvt, 512 * qc:512 * (qc + 1)]
            if bias is not None:
                nc.vector.tensor_scalar_add(out=o, in0=pp, scalar1=bias[:, dvt:dvt + 1])
            else:
                nc.vector.tensor_copy(out=o, in_=pp)
        return emit

    def v_chain(nkt):
        def emit():
            pp = cx.ps_half.tile([128, D], F32, tag="h")
            for kt in range(4):
                nc.tensor.matmul(pp, lhsT=KT[:, kt, 128 * nkt:128 * (nkt + 1)],
                                 rhs=cx.w_v[:, kt, :], start=(kt == 0), stop=(kt == 3))
            if cx.bvb is not None:
                nc.vector.scalar_tensor_tensor(out=vT[:, nkt, :], in0=pp, scalar=0.0,
                                               in1=cx.bvb, op0=OP.add, op1=OP.add)
            else:
                nc.vector.tensor_copy(out=vT[:, nkt, :], in_=pp)
        return emit

    for dvt in range(4):
        for qc in range(2):
            chains.append(qk_chain(qT, cx.w_q, QT, cx.bq4, dvt, qc))
    for dvt in range(4):
        for qc in range(2):
            chains.append(qk_chain(kT, cx.w_k, KT, cx.bk4, dvt, qc))
    for nkt in range(8):
        chains.append(v_chain(nkt))
    return chains


def _p3_attn_unit(nc, cx, qT, kT, vT, XT, SQ, hp, qc, filler=None, fill_at=()):
    ps_u = cx.ps_pv.tile([128, D], F32, tag="u")
    ps_z = cx.ps_z.tile([128, D], F32, tag="z")

    def scores(kt):
        ps_s = cx.ps_wide.tile([128, 2 * D], F32, tag="w")
        nc.tensor.matmul(
            ps_s[:, 0:D],
            lhsT=kT[0:64, hp, 128 * kt:128 * (kt + 1)],
            rhs=qT[0:64, hp, 512 * qc:512 * (qc + 1)],
            start=True, stop=True, tile_position=(0, 0))
        nc.tensor.matmul(
            ps_s[:, D:2 * D],
            lhsT=kT[64:128, hp, 128 * kt:128 * (kt + 1)],
            rhs=qT[64:128, hp, 512 * qc:512 * (qc + 1)],
            start=True, stop=True, tile_position=(64, 0))
        return ps_s

    # Emit scores one kt ahead of the exp->Z/PV consumers: each engine's
    # stream executes in program order, so PV(kt) (gated on exp(kt)) must
    # not block the independent scores(kt+1) on PE.
    ps_s_next = scores(0)
    for kt in range(8):
        ps_s = ps_s_next
        if kt < 7:
            ps_s_next = scores(kt + 1)
        ex = cx.p_ex.tile([128, 2 * D], BF16)
        nc.scalar.activation(out=ex, in_=ps_s, func=AF.Exp, scale=SCALE)
        nc.tensor.matmul(ps_z[0:64, :], lhsT=cx.ones64, rhs=ex[:, 0:D],
                         start=(kt == 0), stop=(kt == 7), tile_position=(0, 0))
        nc.tensor.matmul(ps_z[64:128, :], lhsT=cx.ones64, rhs=ex[:, D:2 * D],
                         start=(kt == 0), stop=(kt == 7), tile_position=(0, 64))
        nc.tensor.matmul(ps_u[0:64, :],
                         lhsT=vT[:, kt, 128 * hp:128 * hp + 64],
                         rhs=ex[:, 0:D],
                         start=(kt == 0), stop=(kt == 7), tile_position=(0, 0))
        nc.tensor.matmul(ps_u[64:128, :],
                         lhsT=vT[:, kt, 128 * hp + 64:128 * (hp + 1)],
                         rhs=ex[:, D:2 * D],
                         start=(kt == 0), stop=(kt == 7), tile_position=(0, 64))
        if filler and kt in fill_at:
            filler.popleft()()
    # ps_z rows 0:64 all hold Z(head 2hp), rows 64:128 all Z(head 2hp+1)
    # (ones64 lhsT, same N-bound matmul cost as an M=1 reduction), so the
    # reciprocal is already partition-broadcast.
    rz = cx.p_rzb.tile([128, D], BF16, tag="rzb")
    with nc.allow_low_precision(reason="softmax 1/Z in bf16 is consistent with bf16 probs"):
        nc.vector.reciprocal(out=rz, in_=ps_z)
    t1 = cx.p_t1.tile([128, D], BF16, tag="t")
    nc.vector.tensor_tensor(out=t1, in0=ps_u, in1=rz, op=OP.mult)
    xs = XT[:, hp, 512 * qc:512 * (qc + 1)]
    nc.vector.tensor_tensor(out=xs, in0=t1,
                            in1=qT[:, hp, 512 * qc:512 * (qc + 1)], op=OP.add)
    nc.vector.tensor_tensor(out=SQ[:, hp, 512 * qc:512 * (qc + 1)],
                            in0=xs, in1=xs, op=OP.mult)


def _p4_ln0_qc(nc, cx, XT, SQ, ln0_aff, XnT, qc):
    """LN0 stats + normalize for one q-half; per-qc small-tile tags so the
    two halves' scalar chains don't serialize through shared slots."""
    ps_st = cx.ps_half.tile([128, D], F32, tag="h")
    for dvt in range(4):
        nc.tensor.matmul(ps_st[0:1, :], lhsT=cx.onesc,
                         rhs=XT[:, dvt, 512 * qc:512 * (qc + 1)],
                         start=(dvt == 0), stop=(dvt == 3), tile_position=(0, 0))
        nc.tensor.matmul(ps_st[32:33, :], lhsT=cx.onesc,
                         rhs=SQ[:, dvt, 512 * qc:512 * (qc + 1)],
                         start=(dvt == 0), stop=(dvt == 3), tile_position=(0, 32))
    mu = cx.p_sml.tile([1, D], F32, tag=f"mu{qc}")
    nc.vector.tensor_scalar_mul(out=mu, in0=ps_st[0:1, :], scalar1=1.0 / D)
    mu2 = cx.p_sml.tile([1, D], BF16, tag=f"tmp{qc}")
    nc.vector.tensor_tensor(out=mu2, in0=mu, in1=mu, op=OP.mult)
    var = cx.p_sml.tile([1, D], F32, tag=f"var{qc}")
    nc.vector.scalar_tensor_tensor(out=var, in0=ps_st[32:33, :],
                                   scalar=1.0 / D, in1=mu2,
                                   op0=OP.mult, op1=OP.subtract)
    lnv = cx.p_sml.tile([1, D], BF16, tag=f"tmp{qc}")
    nc.scalar.activation(out=lnv, in_=var, func=AF.Ln, bias=cx.eps1, scale=1.0)
    rstd = cx.p_sml.tile([1, D], BF16, tag=f"rstd{qc}")
    nc.scalar.activation(out=rstd, in_=lnv, func=AF.Exp, scale=-0.5)
    nmr = cx.p_sml.tile([1, D], BF16, tag=f"nmr{qc}")
    nc.vector.scalar_tensor_tensor(out=nmr, in0=mu, scalar=-1.0, in1=rstd,
                                   op0=OP.mult, op1=OP.mult)
    rstdb = cx.p_rzb.tile([128, D], BF16, tag="rstdb")
    nc.gpsimd.partition_broadcast(rstdb, rstd)
    nmrb = cx.p_rzb.tile([128, D], BF16, tag="nmrb")
    nc.gpsimd.partition_broadcast(nmrb, nmr)
    for dvt in range(4):
        t2 = cx.p_t1.tile([128, D], BF16, tag="t")
        nc.vector.tensor_tensor(out=t2, in0=XT[:, dvt, 512 * qc:512 * (qc + 1)],
                                in1=rstdb, op=OP.mult)
        xn = XnT[:, dvt, 512 * qc:512 * (qc + 1)]
        nc.vector.tensor_tensor(out=xn, in0=t2, in1=nmrb, op=OP.add)
        if ln0_aff:
            nc.vector.tensor_scalar(out=xn, in0=xn,
                                    scalar1=cx.g04[:, dvt:dvt + 1],
                                    scalar2=cx.b04[:, dvt:dvt + 1],
                                    op0=OP.mult, op1=OP.add)


def _p4_ln0(nc, cx, XT, SQ, ln0_aff):
    XnT = cx.p_xnt.tile([128, 4, NQ], BF16)
    for qc in range(2):
        _p4_ln0_qc(nc, cx, XT, SQ, ln0_aff, XnT, qc)
    return XnT


class _P5State:
    pass


def _p5_start(nc, cx, XnT):
    # Xn natural [nq, dv] via DMA xbar transpose: out[p, nqt, dvt, c] with
    # nq = nqt*128 + p, dv = dvt*128 + c.  ACT-issued: separate HWDGE queue
    # from the P1 input transposes on SP.
    st = _P5State()
    st.XnT = XnT
    st.Xn = cx.p_xn.tile([128, 8, 4, 128], BF16, tag="Xn")
    for dvt in range(4):
        nc.scalar.dma_start_transpose(out=st.Xn[:, :, dvt, :], in_=XnT[:, dvt, :])
    st.xpre_l, st.mv_l = [], []
    st.vars8 = cx.p_sml.tile([128, 8], F32, tag="vars8")
    return st


def _p5_chunk(nc, cx, st, nqt):
    XnT, Xn = st.XnT, st.Xn
    ps_m = cx.ps_half.tile([128, D], F32, tag="h")
    for dvt in range(4):
        nc.tensor.matmul(ps_m, lhsT=XnT[:, dvt, 128 * nqt:128 * (nqt + 1)],
                         rhs=cx.w_o[:, dvt, :],
                         start=(dvt == 0), stop=(dvt == 3))
    rl = cx.p_t1.tile([128, D], BF16, tag="t")
    if cx.bob is not None:
        tb = cx.p_t1.tile([128, D], BF16, tag="t")
        nc.vector.tensor_tensor(out=tb, in0=cx.bob, in1=ps_m, op=OP.add)
        nc.vector.tensor_scalar_max(out=rl, in0=tb, scalar1=0.0)
    else:
        nc.vector.tensor_scalar_max(out=rl, in0=ps_m, scalar1=0.0)
    xpre = cx.p_xp.tile([128, D], F32)
    nc.vector.tensor_tensor(out=xpre, in0=rl,
                            in1=Xn[:, nqt, :, :].rearrange("p a b -> p (a b)"),
                            op=OP.add)
    bst = cx.p_bst.tile([128, 6], F32, tag="bst")
    nc.vector.bn_stats(out=bst, in_=xpre)
    mv = cx.p_mv.tile([128, 2], F32, tag="mv")
    nc.vector.bn_aggr(out=mv, in_=bst)
    nc.vector.tensor_copy(out=st.vars8[:, nqt:nqt + 1], in_=mv[:, 1:2])
    st.xpre_l.append(xpre); st.mv_l.append(mv)


def _p5_finish(nc, cx, st, dOut, rb, ln1_aff):
    lnv8 = cx.p_sml.tile([128, 8], F32, tag="lnv8")
    nc.scalar.activation(out=lnv8, in_=st.vars8, func=AF.Ln, bias=cx.epsP, scale=1.0)
    rstd8 = cx.p_sml.tile([128, 8], F32, tag="rstd8")
    nc.scalar.activation(out=rstd8, in_=lnv8, func=AF.Exp, scale=-0.5)
    for g in range(2):
        out4 = cx.p_out.tile([128, 4, D], F32)
        for c in range(4):
            nqt = 4 * g + c
            ot = out4[:, c, :]
            nc.vector.tensor_scalar(out=ot, in0=st.xpre_l[nqt],
                                    scalar1=st.mv_l[nqt][:, 0:1],
                                    scalar2=rstd8[:, nqt:nqt + 1],
                                    op0=OP.subtract, op1=OP.mult)
            if ln1_aff:
                nc.vector.tensor_tensor(out=ot, in0=ot, in1=cx.g1b, op=OP.mult)
                nc.vector.tensor_tensor(out=ot, in0=ot, in1=cx.b1b, op=OP.add)
        nc.scalar.dma_start(
            out=dOut[rb + 512 * g: rb + 512 * (g + 1), :].rearrange(
                "(c p) d -> p c d", p=128),
            in_=out4)


def _build(flags, repeat=1):
    (bq_nz, bk_nz, bv_nz, bo_nz, ln0_aff, ln1_aff) = flags
    nc = bacc.Bacc("TRN2", target_bir_lowering=False, debug=False,
                   num_devices=N_CORES)

    dQ = nc.dram_tensor("Qs", [BL * NQ, D], BF16, kind="ExternalInput").ap()
    dK = nc.dram_tensor("Ks", [BL * NK, D], BF16, kind="ExternalInput").ap()
    dOut = nc.dram_tensor("OUT", [BL * NQ, D], F32, kind="ExternalOutput").ap()

    cx = _Ctx()
    with ExitStack() as es:
        tc = es.enter_context(tile.TileContext(nc))
        ec = es.enter_context
        cst = ec(tc.tile_pool(name="cst", bufs=1))
        cx.p_qkt = ec(tc.tile_pool(name="qkt", bufs=1))
        cx.p_proj = ec(tc.tile_pool(name="proj", bufs=2))
        cx.p_xt = ec(tc.tile_pool(name="xt", bufs=2))
        cx.p_xnt = ec(tc.tile_pool(name="xnt", bufs=2))
        cx.p_xn = ec(tc.tile_pool(name="xn", bufs=1))
        cx.p_ex = ec(tc.tile_pool(name="ex", bufs=3))
        cx.p_rzb = ec(tc.tile_pool(name="rzb", bufs=2))
        cx.p_t1 = ec(tc.tile_pool(name="t1", bufs=3))
        cx.p_xp = ec(tc.tile_pool(name="xp", bufs=9))
        cx.p_out = ec(tc.tile_pool(name="outp", bufs=1))
        cx.p_sml = ec(tc.tile_pool(name="sml", bufs=1))
        cx.p_mv = ec(tc.tile_pool(name="mv", bufs=10))
        cx.p_bst = ec(tc.tile_pool(name="bst", bufs=2))
        cx.ps_wide = ec(tc.tile_pool(name="wide", bufs=2, space="PSUM"))
        cx.ps_half = ec(tc.tile_pool(name="half", bufs=2, space="PSUM"))
        cx.ps_pv = ec(tc.tile_pool(name="pv", bufs=1, space="PSUM"))
        cx.ps_z = ec(tc.tile_pool(name="z", bufs=1, space="PSUM"))
        _setup_consts(nc, cx, cst, flags)

        def body():
            from collections import deque
            units = [(hp, qc) for hp in range(4) for qc in range(2)]
            # Batch 0 inputs, then batch 1 inputs (all DMA-only, queue early)
            QT0 = _p1_transpose(nc, cx, 0, dQ, "QT")
            KT0 = _p1_transpose(nc, cx, 0, dK, "KT")
            proj0 = _p2_alloc(cx)
            for c in _p2_chains(nc, cx, proj0, QT0, KT0):
                c()
            QT1 = _p1_transpose(nc, cx, NQ, dQ, "QT")
            KT1 = _p1_transpose(nc, cx, NQ, dK, "KT")
            qT0, kT0, vT0 = proj0
            # P3 batch 0, with batch-1 projection chains woven in as PE
            # filler (P3 is ACT-bound; each engine stream runs in program
            # order, so filler must be emitted inline).
            proj1 = _p2_alloc(cx)
            XT0 = cx.p_xt.tile([128, 4, NQ], BF16, tag="XT")
            SQ0 = cx.p_xt.tile([128, 4, NQ], BF16, tag="SQ")
            XnT0 = cx.p_xnt.tile([128, 4, NQ], BF16)

            def f_p4_qc0():
                # qc0 columns of XT0/SQ0 are complete after unit 6 (3,0),
                # so this can run during the last (qc=1) unit of P3_b0.
                _p4_ln0_qc(nc, cx, XT0, SQ0, ln0_aff, XnT0, 0)

            fill = deque(_p2_chains(nc, cx, proj1, QT1, KT1) + [f_p4_qc0])
            for u, (hp, qc) in enumerate(units):
                _p3_attn_unit(nc, cx, qT0, kT0, vT0, XT0, SQ0, hp, qc,
                              filler=fill, fill_at=(2, 4, 6) if u < 7 else (2, 4, 6, 7))
            qT1, kT1, vT1 = proj1
            # P3 batch 1, with batch-0 epilogue (P4 qc1 + P5) woven in.
            XT1 = cx.p_xt.tile([128, 4, NQ], BF16, tag="XT")
            SQ1 = cx.p_xt.tile([128, 4, NQ], BF16, tag="SQ")
            st0_box = []

            def f_p4_qc1():
                _p4_ln0_qc(nc, cx, XT0, SQ0, ln0_aff, XnT0, 1)
                st0_box.append(_p5_start(nc, cx, XnT0))

            def f_chunk(nqt):
                return lambda: _p5_chunk(nc, cx, st0_box[0], nqt)

            def f_finish():
                _p5_finish(nc, cx, st0_box[0], dOut, 0, ln1_aff)

            fill = deque([f_p4_qc1] + [f_chunk(i) for i in range(8)] + [f_finish])
            fills = {0: (2,), 1: (2, 5), 2: (2, 5), 3: (2, 5), 4: (2, 5), 5: (2, 5)}
            for u, (hp, qc) in enumerate(units):
                _p3_attn_unit(nc, cx, qT1, kT1, vT1, XT1, SQ1, hp, qc,
                              filler=fill, fill_at=fills.get(u, ()))
            while fill:
                fill.popleft()()
            # Batch 1 epilogue (tail)
            XnT1 = _p4_ln0(nc, cx, XT1, SQ1, ln0_aff)
            st1 = _p5_start(nc, cx, XnT1)
            for nqt in range(8):
                _p5_chunk(nc, cx, st1, nqt)
            _p5_finish(nc, cx, st1, dOut, NQ, ln1_aff)

        if repeat == 1:
            body()
        else:
            with tc.For_i(0, repeat, 1):
                body()

    nc.compile()
    return nc


def _consts(Wq, Wk, Wv, Wo, flags, bq, bk, bv, bo, g0, b0, g1, b1):
    (bq_nz, bk_nz, bv_nz, bo_nz, ln0_aff, ln1_aff) = flags
    c = {
        "Wqb": np.ascontiguousarray(np.asarray(Wq).astype(NBF)),
        "Wkb": np.ascontiguousarray(np.asarray(Wk).astype(NBF)),
        "Wvb": np.ascontiguousarray(np.asarray(Wv).astype(NBF)),
        "Wob": np.ascontiguousarray(np.asarray(Wo).astype(NBF)),
        "onesc": np.ones((128, 1), NBF),
        "ones64": np.ones((128, 64), NBF),
    }
    if bq_nz: c["bq4"] = np.ascontiguousarray(np.asarray(bq).reshape(4, 128).T.astype(np.float32))
    if bk_nz: c["bk4"] = np.ascontiguousarray(np.asarray(bk).reshape(4, 128).T.astype(np.float32))
    if bv_nz: c["bvb"] = np.ascontiguousarray(np.broadcast_to(np.asarray(bv, np.float32), (128, D)))
    if bo_nz: c["bob"] = np.ascontiguousarray(np.broadcast_to(np.asarray(bo, np.float32), (128, D)))
    if ln0_aff:
        c["g04"] = np.ascontiguousarray(np.asarray(g0).reshape(4, 128).T.astype(np.float32))
        c["b04"] = np.ascontiguousarray(np.asarray(b0).reshape(4, 128).T.astype(np.float32))
    if ln1_aff:
        c["g1b"] = np.ascontiguousarray(np.broadcast_to(np.asarray(g1, np.float32), (128, D)))
        c["b1b"] = np.ascontiguousarray(np.broadcast_to(np.asarray(b1, np.float32), (128, D)))
    return c


def make_in_maps(Q, K, Wq, bq, Wk, bk, Wv, bv, Wo, bo, g0, b0, g1, b1, flags):
    consts = _consts(Wq, Wk, Wv, Wo, flags, bq, bk, bv, bo, g0, b0, g1, b1)
    in_maps = []
    for ci in range(N_CORES):
        m = dict(consts)
        m["Qs"] = np.ascontiguousarray(
            np.asarray(Q)[ci * BL:(ci + 1) * BL].reshape(BL * NQ, D).astype(NBF))
        m["Ks"] = np.ascontiguousarray(
            np.asarray(K)[ci * BL:(ci + 1) * BL].reshape(BL * NK, D).astype(NBF))
        in_maps.append(m)
    return in_maps


def get_flags(bq, bk, bv, bo, g0, b0, g1, b1):
    return (bool(np.any(np.asarray(bq))), bool(np.any(np.asarray(bk))),
            bool(np.any(np.asarray(bv))), bool(np.any(np.asarray(bo))),
            bool(np.any(np.asarray(g0) != 1) or np.any(np.asarray(b0))),
            bool(np.any(np.asarray(g1) != 1) or np.any(np.asarray(b1))))


def get_program(flags, repeat=1):
    key = (flags, repeat)
    if key not in _cache:
        _cache[key] = _build(flags, repeat)
    return _cache[key]


def kernel(Q, K, Wq, bq, Wk, bk, Wv, bv, Wo, bo, g0, b0, g1, b1):
    flags = get_flags(bq, bk, bv, bo, g0, b0, g1, b1)
    nc = get_program(flags, repeat=1)
    in_maps = make_in_maps(Q, K, Wq, bq, Wk, bk, Wv, bv, Wo, bo, g0, b0, g1, b1, flags)
    res = run_bass_kernel_spmd(nc, in_maps, list(range(N_CORES)))
    out = np.empty((B, NQ, D), np.float32)
    for ci in range(N_CORES):
        out[ci * BL:(ci + 1) * BL] = res.results[ci]["OUT"].reshape(BL, NQ, D)
    return out


# revision 56
# speedup vs baseline: 3968.4871x; 1.1444x over previous
"""Trainium2 Bass kernel for MAB (multihead attention block) — nn_MAB_48412871360901.

Data-parallel over batch: 16 batches -> 8 NeuronCores, 2 batches/core.
Per core, per batch (all matmuls bf16 with fp32 PSUM accumulation):
  P1  Q,K loaded natural (batched 4-chunk DMAs), cast bf16, transposed to
      QT/KT [dv, nq] layout via DMA xbar transpose (no PE involvement)
  P2  projections qT = Wq^T QT, kT = Wk^T KT (transposed layout), v = K Wv (natural)
  P3  per (head-pair, q-chunk): S^T = k^T.T q^T (row-packed 2 heads),
      exp on ACT (scale 1/sqrt(512) fused), softmax denominators Z via
      ones-matmuls, PV U^T = v^T expS^T (col-packed 2 heads),
      X^T = U^T * (1/Z)bcast + qT   (residual uses post-projection q)
  P4  LN0 in transposed layout: stats via ones-matmuls over partitions,
      rstd = exp(-0.5 ln(var+eps)) on ACT, normalize with PE-broadcast tiles
  P5  Xn natural obtained via DMA xbar transpose of XnT; M = Xn Wo (natural
      out from XnT lhsT); relu; residual; LN1 natural (bn_stats); batched
      DMA out fp32.

All scalar-engine activations are Exp/Ln, forced into the single
natural_log_exp_and_others table set to avoid ACT table-load thrash.
"""

import sys
import functools
from contextlib import ExitStack
import numpy as np
import ml_dtypes

for _p in ("/opt/trn_rl_repo", "/root/.axon_site/_ro/trn_rl_repo"):
    if _p not in sys.path:
        sys.path.insert(0, _p)

import concourse.bacc as bacc
import concourse.mybir as mybir
import concourse.tile as tile
from concourse.bass_utils import run_bass_kernel_spmd
from concourse.hw_specs import get_activation_tables as _orig_gat

BF16 = mybir.dt.bfloat16
F32 = mybir.dt.float32
NBF = ml_dtypes.bfloat16
AF = mybir.ActivationFunctionType
OP = mybir.AluOpType

B, NQ, NK = 16, 1024, 1024
D = 512
H = 8
N_CORES = 8
BL = B // N_CORES          # batches per core
EPS = 1e-5
SCALE = 1.0 / np.sqrt(512.0)

_ONE_SET = "natural_log_exp_and_others"


@functools.cache
def _gat_one_set(arch):
    """Empty out every activation-table set except the one containing both
    Exp and Ln, so bacc's table-load pass emits a single LoadActFuncSet
    instead of thrashing between exp_and_others and natural_log.
    Set indices (act_func_set_id) are preserved."""
    tabs = _orig_gat(arch)
    return {name: (fns if name == _ONE_SET else frozenset())
            for name, fns in tabs.items()}


bacc.get_activation_tables = _gat_one_set

_cache = {}


class _Ctx:
    pass


def _setup_consts(nc, cx, cst, flags):
    (bq_nz, bk_nz, bv_nz, bo_nz, ln0_aff, ln1_aff) = flags

    def din(name, shape, dt=BF16):
        return nc.dram_tensor(name, list(shape), dt, kind="ExternalInput").ap()

    def ldc(name, dshape, shape, rearr=None):
        d = din(name, dshape)
        t = cst.tile(list(shape), BF16, tag=name)
        nc.sync.dma_start(out=t, in_=d if rearr is None else d.rearrange(rearr, p=128))
        return t

    def ldf(name, shape):
        d = din(name, shape, F32)
        t = cst.tile(list(shape), F32, tag=name)
        nc.sync.dma_start(out=t, in_=d)
        return t

    cx.w_q = ldc("Wqb", (D, D), (128, 4, D), "(kt p) c -> p kt c")
    cx.w_k = ldc("Wkb", (D, D), (128, 4, D), "(kt p) c -> p kt c")
    cx.w_v = ldc("Wvb", (D, D), (128, 4, D), "(kt p) c -> p kt c")
    cx.w_o = ldc("Wob", (D, D), (128, 4, D), "(kt p) c -> p kt c")
    cx.onesc = ldc("onesc", (128, 1), (128, 1))
    cx.ones64 = ldc("ones64", (128, 64), (128, 64))
    cx.epsP = cst.tile([128, 1], F32, tag="epsP"); nc.vector.memset(cx.epsP, EPS)
    cx.eps1 = cst.tile([1, 1], F32, tag="eps1"); nc.vector.memset(cx.eps1, EPS)
    cx.bq4 = ldf("bq4", (128, 4)) if bq_nz else None
    cx.bk4 = ldf("bk4", (128, 4)) if bk_nz else None
    cx.bvb = ldf("bvb", (128, D)) if bv_nz else None
    cx.bob = ldf("bob", (128, D)) if bo_nz else None
    cx.g04 = ldf("g04", (128, 4)) if ln0_aff else None
    cx.b04 = ldf("b04", (128, 4)) if ln0_aff else None
    cx.g1b = ldf("g1b", (128, D)) if ln1_aff else None
    cx.b1b = ldf("b1b", (128, D)) if ln1_aff else None


def _p1_transpose(nc, cx, rb, src_dram, tag):
    """DMA-xbar-transpose bf16 input (host-cast) straight from DRAM into the
    [dv, nq] layout.

    dma_start_transpose maps transposed row r of a [128, 512] input to
    out[p, kt, c] with r = kt*128 + p — the same (kt p) layout the weight
    tiles use, so the projection matmuls consume dst directly."""
    dst = cx.p_qkt.tile([128, 4, NQ], BF16, tag=tag)
    for i in range(8):
        nc.sync.dma_start_transpose(
            out=dst[:, :, 128 * i:128 * (i + 1)],
            in_=src_dram[rb + 128 * i: rb + 128 * (i + 1), :])
    return dst


def _p2_alloc(cx):
    qT = cx.p_proj.tile([128, 4, NQ], BF16, tag="qT")
    kT = cx.p_proj.tile([128, 4, NQ], BF16, tag="kT")
    vT = cx.p_proj.tile([128, 8, D], BF16, tag="vT")
    return qT, kT, vT


def _p2_chains(nc, cx, proj, QT, KT):
    """Return the 24 projection chains (4 PE matmuls + evac each) as
    closures, so the body can interleave their EMISSION into other phases —
    each engine's instruction stream executes in program order, so filler
    work must be woven in at emission time."""
    qT, kT, vT = proj
    chains = []

    def qk_chain(dst, w, srcT, bias, dvt, qc):
        def emit():
            pp = cx.ps_half.tile([128, D], F32, tag="h")
            for kt in range(4):
                nc.tensor.matmul(
                    pp, lhsT=w[:, kt, 128 * dvt:128 * (dvt + 1)],
                    rhs=srcT[:, kt, 512 * qc:512 * (qc + 1)],
                    start=(kt == 0), stop=(kt == 3))
            o = dst[:, dvt, 512 * qc:512 * (qc + 1)]
            if bias is not None:
                nc.vector.tensor_scalar_add(out=o, in0=pp, scalar1=bias[:, dvt:dvt + 1])
            else:
                nc.vector.tensor_copy(out=o, in_=pp)
        return emit

    def v_chain(nkt):
        def emit():
            pp = cx.ps_half.tile([128, D], F32, tag="h")
            for kt in range(4):
                nc.tensor.matmul(pp, lhsT=KT[:, kt, 128 * nkt:128 * (nkt + 1)],
                                 rhs=cx.w_v[:, kt, :], start=(kt == 0), stop=(kt == 3))
            if cx.bvb is not None:
                nc.vector.scalar_tensor_tensor(out=vT[:, nkt, :], in0=pp, scalar=0.0,
                                               in1=cx.bvb, op0=OP.add, op1=OP.add)
            else:
                nc.vector.tensor_copy(out=vT[:, nkt, :], in_=pp)
        return emit

    for dvt in range(4):
        for qc in range(2):
            chains.append(qk_chain(qT, cx.w_q, QT, cx.bq4, dvt, qc))
    for dvt in range(4):
        for qc in range(2):
            chains.append(qk_chain(kT, cx.w_k, KT, cx.bk4, dvt, qc))
    for nkt in range(8):
        chains.append(v_chain(nkt))
    return chains


def _p3_batch(nc, cx, qT, kT, vT, XT, SQ, filler=None, fill_steps=()):
    """All 8 attention units of one batch as a single software-pipelined
    stream: the score matmuls run one (unit, kt) step ahead of the
    exp->Z/PV consumers ACROSS unit boundaries, so the in-order PE stream
    never stalls on the current step's exp."""
    from collections import deque
    units = [(hp, qc) for hp in range(4) for qc in range(2)]

    def scores(hp, qc, kt):
        ps_s = cx.ps_wide.tile([128, 2 * D], F32, tag="w")
        nc.tensor.matmul(
            ps_s[:, 0:D],
            lhsT=kT[0:64, hp, 128 * kt:128 * (kt + 1)],
            rhs=qT[0:64, hp, 512 * qc:512 * (qc + 1)],
            start=True, stop=True, tile_position=(0, 0))
        nc.tensor.matmul(
            ps_s[:, D:2 * D],
            lhsT=kT[64:128, hp, 128 * kt:128 * (kt + 1)],
            rhs=qT[64:128, hp, 512 * qc:512 * (qc + 1)],
            start=True, stop=True, tile_position=(64, 0))
        return ps_s

    steps = [(u, kt) for u in range(8) for kt in range(8)]
    sq = deque()
    cur = {}
    sq.append(scores(*units[0], 0))
    for i, (u, kt) in enumerate(steps):
        hp, qc = units[u]
        if kt == 0:
            ps_u_new = cx.ps_pv.tile([128, D], F32, tag="u")
            ps_z_new = cx.ps_z.tile([128, D], F32, tag="z")
            cur[u] = (ps_u_new, ps_z_new)
        ps_u, ps_z = cur[u]
        if i + 1 < len(steps):
            un, ktn = steps[i + 1]
            sq.append(scores(*units[un], ktn))
        ps_s = sq.popleft()
        ex = cx.p_ex.tile([128, 2 * D], BF16)
        nc.scalar.activation(out=ex, in_=ps_s, func=AF.Exp, scale=SCALE)
        nc.tensor.matmul(ps_z[0:64, :], lhsT=cx.ones64, rhs=ex[:, 0:D],
                         start=(kt == 0), stop=(kt == 7), tile_position=(0, 0))
        nc.tensor.matmul(ps_z[64:128, :], lhsT=cx.ones64, rhs=ex[:, D:2 * D],
                         start=(kt == 0), stop=(kt == 7), tile_position=(0, 64))
        nc.tensor.matmul(ps_u[0:64, :],
                         lhsT=vT[:, kt, 128 * hp:128 * hp + 64],
                         rhs=ex[:, 0:D],
                         start=(kt == 0), stop=(kt == 7), tile_position=(0, 0))
        nc.tensor.matmul(ps_u[64:128, :],
                         lhsT=vT[:, kt, 128 * hp + 64:128 * (hp + 1)],
                         rhs=ex[:, D:2 * D],
                         start=(kt == 0), stop=(kt == 7), tile_position=(0, 64))
        if filler and i in fill_steps:
            filler.popleft()()
        if kt == 7:
            # Unit epilogue.  ps_z rows 0:64 all hold Z(head 2hp), rows
            # 64:128 all Z(head 2hp+1) (ones64 lhsT, same N-bound matmul
            # cost as an M=1 reduction), so the reciprocal is already
            # partition-broadcast.
            rz = cx.p_rzb.tile([128, D], BF16, tag="rzb")
            with nc.allow_low_precision(reason="softmax 1/Z in bf16"):
                nc.vector.reciprocal(out=rz, in_=ps_z)
            t1 = cx.p_t1.tile([128, D], BF16, tag="t")
            nc.vector.tensor_tensor(out=t1, in0=ps_u, in1=rz, op=OP.mult)
            xs = XT[:, hp, 512 * qc:512 * (qc + 1)]
            nc.vector.tensor_tensor(out=xs, in0=t1,
                                    in1=qT[:, hp, 512 * qc:512 * (qc + 1)],
                                    op=OP.add)
            nc.vector.tensor_tensor(out=SQ[:, hp, 512 * qc:512 * (qc + 1)],
                                    in0=xs, in1=xs, op=OP.mult)
            del cur[u]


def _p4_ln0_qc(nc, cx, XT, SQ, ln0_aff, XnT, qc):
    """LN0 stats + normalize for one q-half; per-qc small-tile tags so the
    two halves' scalar chains don't serialize through shared slots."""
    ps_st = cx.ps_half.tile([128, D], F32, tag="h")
    for dvt in range(4):
        nc.tensor.matmul(ps_st[0:1, :], lhsT=cx.onesc,
                         rhs=XT[:, dvt, 512 * qc:512 * (qc + 1)],
                         start=(dvt == 0), stop=(dvt == 3), tile_position=(0, 0))
        nc.tensor.matmul(ps_st[32:33, :], lhsT=cx.onesc,
                         rhs=SQ[:, dvt, 512 * qc:512 * (qc + 1)],
                         start=(dvt == 0), stop=(dvt == 3), tile_position=(0, 32))
    mu = cx.p_sml.tile([1, D], F32, tag=f"mu{qc}")
    nc.vector.tensor_scalar_mul(out=mu, in0=ps_st[0:1, :], scalar1=1.0 / D)
    mu2 = cx.p_sml.tile([1, D], BF16, tag=f"tmp{qc}")
    nc.vector.tensor_tensor(out=mu2, in0=mu, in1=mu, op=OP.mult)
    var = cx.p_sml.tile([1, D], F32, tag=f"var{qc}")
    nc.vector.scalar_tensor_tensor(out=var, in0=ps_st[32:33, :],
                                   scalar=1.0 / D, in1=mu2,
                                   op0=OP.mult, op1=OP.subtract)
    lnv = cx.p_sml.tile([1, D], BF16, tag=f"tmp{qc}")
    nc.scalar.activation(out=lnv, in_=var, func=AF.Ln, bias=cx.eps1, scale=1.0)
    rstd = cx.p_sml.tile([1, D], BF16, tag=f"rstd{qc}")
    nc.scalar.activation(out=rstd, in_=lnv, func=AF.Exp, scale=-0.5)
    nmr = cx.p_sml.tile([1, D], BF16, tag=f"nmr{qc}")
    nc.vector.scalar_tensor_tensor(out=nmr, in0=mu, scalar=-1.0, in1=rstd,
                                   op0=OP.mult, op1=OP.mult)
    rstdb = cx.p_rzb.tile([128, D], BF16, tag="rstdb")
    nc.gpsimd.partition_broadcast(rstdb, rstd)
    nmrb = cx.p_rzb.tile([128, D], BF16, tag="nmrb")
    nc.gpsimd.partition_broadcast(nmrb, nmr)
    for dvt in range(4):
        t2 = cx.p_t1.tile([128, D], BF16, tag="t")
        nc.vector.tensor_tensor(out=t2, in0=XT[:, dvt, 512 * qc:512 * (qc + 1)],
                                in1=rstdb, op=OP.mult)
        xn = XnT[:, dvt, 512 * qc:512 * (qc + 1)]
        nc.vector.tensor_tensor(out=xn, in0=t2, in1=nmrb, op=OP.add)
        if ln0_aff:
            nc.vector.tensor_scalar(out=xn, in0=xn,
                                    scalar1=cx.g04[:, dvt:dvt + 1],
                                    scalar2=cx.b04[:, dvt:dvt + 1],
                                    op0=OP.mult, op1=OP.add)


def _p4_ln0(nc, cx, XT, SQ, ln0_aff):
    XnT = cx.p_xnt.tile([128, 4, NQ], BF16)
    for qc in range(2):
        _p4_ln0_qc(nc, cx, XT, SQ, ln0_aff, XnT, qc)
    return XnT


class _P5State:
    pass


def _p5_start(nc, cx, XnT):
    # Xn natural [nq, dv] via DMA xbar transpose: out[p, nqt, dvt, c] with
    # nq = nqt*128 + p, dv = dvt*128 + c.  ACT-issued: separate HWDGE queue
    # from the P1 input transposes on SP.
    st = _P5State()
    st.XnT = XnT
    st.Xn = cx.p_xn.tile([128, 8, 4, 128], BF16, tag="Xn")
    for dvt in range(4):
        nc.scalar.dma_start_transpose(out=st.Xn[:, :, dvt, :], in_=XnT[:, dvt, :])
    st.xpre_l, st.mv_l = [], []
    vars4_0 = cx.p_sml.tile([128, 4], F32, tag="vars4_0")
    vars4_1 = cx.p_sml.tile([128, 4], F32, tag="vars4_1")
    st.vars4 = [vars4_0, vars4_1]
    return st


def _p5_chunk(nc, cx, st, nqt):
    XnT, Xn = st.XnT, st.Xn
    ps_m = cx.ps_half.tile([128, D], F32, tag="h")
    for dvt in range(4):
        nc.tensor.matmul(ps_m, lhsT=XnT[:, dvt, 128 * nqt:128 * (nqt + 1)],
                         rhs=cx.w_o[:, dvt, :],
                         start=(dvt == 0), stop=(dvt == 3))
    rl = cx.p_t1.tile([128, D], BF16, tag="t")
    if cx.bob is not None:
        tb = cx.p_t1.tile([128, D], BF16, tag="t")
        nc.vector.tensor_tensor(out=tb, in0=cx.bob, in1=ps_m, op=OP.add)
        nc.vector.tensor_scalar_max(out=rl, in0=tb, scalar1=0.0)
    else:
        nc.vector.tensor_scalar_max(out=rl, in0=ps_m, scalar1=0.0)
    xpre = cx.p_xp.tile([128, D], F32)
    nc.vector.tensor_tensor(out=xpre, in0=rl,
                            in1=Xn[:, nqt, :, :].rearrange("p a b -> p (a b)"),
                            op=OP.add)
    bst = cx.p_bst.tile([128, 6], F32, tag="bst")
    nc.vector.bn_stats(out=bst, in_=xpre)
    mv = cx.p_mv.tile([128, 2], F32, tag="mv")
    nc.vector.bn_aggr(out=mv, in_=bst)
    nc.vector.tensor_copy(out=st.vars4[nqt // 4][:, nqt % 4:nqt % 4 + 1],
                          in_=mv[:, 1:2])
    st.xpre_l.append(xpre); st.mv_l.append(mv)


def _p5_finish_g(nc, cx, st, dOut, rb, ln1_aff, g):
    """LN1 rstd + final normalize + store for one group of 4 nq-chunks, so
    group 0's output DMA can overlap group 1's chunks."""
    lnv4 = cx.p_sml.tile([128, 4], F32, tag=f"lnv4_{g}")
    nc.scalar.activation(out=lnv4, in_=st.vars4[g], func=AF.Ln, bias=cx.epsP, scale=1.0)
    rstd4 = cx.p_sml.tile([128, 4], F32, tag=f"rstd4_{g}")
    nc.scalar.activation(out=rstd4, in_=lnv4, func=AF.Exp, scale=-0.5)
    out4 = cx.p_out.tile([128, 4, D], F32)
    for c in range(4):
        nqt = 4 * g + c
        ot = out4[:, c, :]
        nc.vector.tensor_scalar(out=ot, in0=st.xpre_l[nqt],
                                scalar1=st.mv_l[nqt][:, 0:1],
                                scalar2=rstd4[:, c:c + 1],
                                op0=OP.subtract, op1=OP.mult)
        if ln1_aff:
            nc.vector.tensor_tensor(out=ot, in0=ot, in1=cx.g1b, op=OP.mult)
            nc.vector.tensor_tensor(out=ot, in0=ot, in1=cx.b1b, op=OP.add)
    nc.scalar.dma_start(
        out=dOut[rb + 512 * g: rb + 512 * (g + 1), :].rearrange(
            "(c p) d -> p c d", p=128),
        in_=out4)


def _build(flags, repeat=1):
    (bq_nz, bk_nz, bv_nz, bo_nz, ln0_aff, ln1_aff) = flags
    nc = bacc.Bacc("TRN2", target_bir_lowering=False, debug=False,
                   num_devices=N_CORES)

    dQ = nc.dram_tensor("Qs", [BL * NQ, D], BF16, kind="ExternalInput").ap()
    dK = nc.dram_tensor("Ks", [BL * NK, D], BF16, kind="ExternalInput").ap()
    dOut = nc.dram_tensor("OUT", [BL * NQ, D], F32, kind="ExternalOutput").ap()

    cx = _Ctx()
    with ExitStack() as es:
        tc = es.enter_context(tile.TileContext(nc))
        ec = es.enter_context
        cst = ec(tc.tile_pool(name="cst", bufs=1))
        cx.p_qkt = ec(tc.tile_pool(name="qkt", bufs=1))
        cx.p_proj = ec(tc.tile_pool(name="proj", bufs=2))
        cx.p_xt = ec(tc.tile_pool(name="xt", bufs=2))
        cx.p_xnt = ec(tc.tile_pool(name="xnt", bufs=2))
        cx.p_xn = ec(tc.tile_pool(name="xn", bufs=1))
        cx.p_ex = ec(tc.tile_pool(name="ex", bufs=3))
        cx.p_rzb = ec(tc.tile_pool(name="rzb", bufs=2))
        cx.p_t1 = ec(tc.tile_pool(name="t1", bufs=3))
        cx.p_xp = ec(tc.tile_pool(name="xp", bufs=9))
        cx.p_out = ec(tc.tile_pool(name="outp", bufs=1))
        cx.p_sml = ec(tc.tile_pool(name="sml", bufs=1))
        cx.p_mv = ec(tc.tile_pool(name="mv", bufs=10))
        cx.p_bst = ec(tc.tile_pool(name="bst", bufs=2))
        cx.ps_wide = ec(tc.tile_pool(name="wide", bufs=2, space="PSUM"))
        cx.ps_half = ec(tc.tile_pool(name="half", bufs=2, space="PSUM"))
        cx.ps_pv = ec(tc.tile_pool(name="pv", bufs=1, space="PSUM"))
        cx.ps_z = ec(tc.tile_pool(name="z", bufs=1, space="PSUM"))
        _setup_consts(nc, cx, cst, flags)

        def body():
            from collections import deque
            units = [(hp, qc) for hp in range(4) for qc in range(2)]
            # Batch 0 inputs, then batch 1 inputs (all DMA-only, queue early)
            QT0 = _p1_transpose(nc, cx, 0, dQ, "QT")
            KT0 = _p1_transpose(nc, cx, 0, dK, "KT")
            proj0 = _p2_alloc(cx)
            for c in _p2_chains(nc, cx, proj0, QT0, KT0):
                c()
            QT1 = _p1_transpose(nc, cx, NQ, dQ, "QT")
            KT1 = _p1_transpose(nc, cx, NQ, dK, "KT")
            qT0, kT0, vT0 = proj0
            # P3 batch 0, with batch-1 projection chains woven in as PE
            # filler (P3 is ACT-bound; each engine stream runs in program
            # order, so filler must be emitted inline).
            proj1 = _p2_alloc(cx)
            XT0 = cx.p_xt.tile([128, 4, NQ], BF16, tag="XT")
            SQ0 = cx.p_xt.tile([128, 4, NQ], BF16, tag="SQ")
            XnT0 = cx.p_xnt.tile([128, 4, NQ], BF16)

            def f_p4_qc0():
                # qc0 columns of XT0/SQ0 are complete after unit 6 (3,0),
                # so this can run during the last (qc=1) unit of P3_b0.
                _p4_ln0_qc(nc, cx, XT0, SQ0, ln0_aff, XnT0, 0)

            # P2_b1 chains front-loaded (4/unit early, tapering) so PE
            # saturation near the end of P3_b0 doesn't starve the exp
            # pipeline; P4 qc0 last.
            fill = deque(_p2_chains(nc, cx, proj1, QT1, KT1) + [f_p4_qc0])
            fsteps = ({8 * u + k for u in range(4) for k in (1, 3, 5, 7)}
                      | {8 * u + k for u in range(4, 7) for k in (2, 4)}
                      | {58, 60} | {63})
            _p3_batch(nc, cx, qT0, kT0, vT0, XT0, SQ0, filler=fill,
                      fill_steps=fsteps)
            qT1, kT1, vT1 = proj1
            # P3 batch 1, with batch-0 epilogue (P4 qc1 + P5) woven in.
            XT1 = cx.p_xt.tile([128, 4, NQ], BF16, tag="XT")
            SQ1 = cx.p_xt.tile([128, 4, NQ], BF16, tag="SQ")
            XnT1 = cx.p_xnt.tile([128, 4, NQ], BF16)
            st0_box = []

            def f_p4_qc1():
                _p4_ln0_qc(nc, cx, XT0, SQ0, ln0_aff, XnT0, 1)
                st0_box.append(_p5_start(nc, cx, XnT0))

            def f_chunk(nqt):
                return lambda: _p5_chunk(nc, cx, st0_box[0], nqt)

            def f_finish(g):
                return lambda: _p5_finish_g(nc, cx, st0_box[0], dOut, 0, ln1_aff, g)

            def f_p4b1_qc0():
                _p4_ln0_qc(nc, cx, XT1, SQ1, ln0_aff, XnT1, 0)

            fill = deque([f_p4_qc1,
                          f_chunk(0), f_chunk(1), f_chunk(2), f_chunk(3),
                          f_finish(0),
                          f_chunk(4), f_chunk(5), f_chunk(6), f_chunk(7),
                          f_finish(1),
                          f_p4b1_qc0])
            fsteps = ({2} | {8 * u + k for u in range(1, 6) for k in (2, 5)}
                      | {58})
            _p3_batch(nc, cx, qT1, kT1, vT1, XT1, SQ1, filler=fill,
                      fill_steps=fsteps)
            while fill:
                fill.popleft()()
            # Batch 1 epilogue (tail)
            _p4_ln0_qc(nc, cx, XT1, SQ1, ln0_aff, XnT1, 1)
            st1 = _p5_start(nc, cx, XnT1)
            for nqt in range(4):
                _p5_chunk(nc, cx, st1, nqt)
            _p5_finish_g(nc, cx, st1, dOut, NQ, ln1_aff, 0)
            for nqt in range(4, 8):
                _p5_chunk(nc, cx, st1, nqt)
            _p5_finish_g(nc, cx, st1, dOut, NQ, ln1_aff, 1)

        if repeat == 1:
            body()
        else:
            # Branch hints: the body far exceeds one IRAM block per engine,
            # so the back-edge would I$-miss (~4us) without prefetch hints.
            hints = (mybir.EngineType.PE, mybir.EngineType.DVE,
                     mybir.EngineType.Activation, mybir.EngineType.Pool,
                     mybir.EngineType.SP)
            with tc.For_i(0, repeat, 1, hint_engines=hints):
                body()

    nc.compile()
    return nc


def _consts(Wq, Wk, Wv, Wo, flags, bq, bk, bv, bo, g0, b0, g1, b1):
    (bq_nz, bk_nz, bv_nz, bo_nz, ln0_aff, ln1_aff) = flags
    c = {
        "Wqb": np.ascontiguousarray(np.asarray(Wq).astype(NBF)),
        "Wkb": np.ascontiguousarray(np.asarray(Wk).astype(NBF)),
        "Wvb": np.ascontiguousarray(np.asarray(Wv).astype(NBF)),
        "Wob": np.ascontiguousarray(np.asarray(Wo).astype(NBF)),
        "onesc": np.ones((128, 1), NBF),
        "ones64": np.ones((128, 64), NBF),
    }
    if bq_nz: c["bq4"] = np.ascontiguousarray(np.asarray(bq).reshape(4, 128).T.astype(np.float32))
    if bk_nz: c["bk4"] = np.ascontiguousarray(np.asarray(bk).reshape(4, 128).T.astype(np.float32))
    if bv_nz: c["bvb"] = np.ascontiguousarray(np.broadcast_to(np.asarray(bv, np.float32), (128, D)))
    if bo_nz: c["bob"] = np.ascontiguousarray(np.broadcast_to(np.asarray(bo, np.float32), (128, D)))
    if ln0_aff:
        c["g04"] = np.ascontiguousarray(np.asarray(g0).reshape(4, 128).T.astype(np.float32))
        c["b04"] = np.ascontiguousarray(np.asarray(b0).reshape(4, 128).T.astype(np.float32))
    if ln1_aff:
        c["g1b"] = np.ascontiguousarray(np.broadcast_to(np.asarray(g1, np.float32), (128, D)))
        c["b1b"] = np.ascontiguousarray(np.broadcast_to(np.asarray(b1, np.float32), (128, D)))
    return c


def make_in_maps(Q, K, Wq, bq, Wk, bk, Wv, bv, Wo, bo, g0, b0, g1, b1, flags):
    consts = _consts(Wq, Wk, Wv, Wo, flags, bq, bk, bv, bo, g0, b0, g1, b1)
    in_maps = []
    for ci in range(N_CORES):
        m = dict(consts)
        m["Qs"] = np.ascontiguousarray(
            np.asarray(Q)[ci * BL:(ci + 1) * BL].reshape(BL * NQ, D).astype(NBF))
        m["Ks"] = np.ascontiguousarray(
            np.asarray(K)[ci * BL:(ci + 1) * BL].reshape(BL * NK, D).astype(NBF))
        in_maps.append(m)
    return in_maps


def get_flags(bq, bk, bv, bo, g0, b0, g1, b1):
    return (bool(np.any(np.asarray(bq))), bool(np.any(np.asarray(bk))),
            bool(np.any(np.asarray(bv))), bool(np.any(np.asarray(bo))),
            bool(np.any(np.asarray(g0) != 1) or np.any(np.asarray(b0))),
            bool(np.any(np.asarray(g1) != 1) or np.any(np.asarray(b1))))


def get_program(flags, repeat=1):
    key = (flags, repeat)
    if key not in _cache:
        _cache[key] = _build(flags, repeat)
    return _cache[key]


def kernel(Q, K, Wq, bq, Wk, bk, Wv, bv, Wo, bo, g0, b0, g1, b1):
    flags = get_flags(bq, bk, bv, bo, g0, b0, g1, b1)
    nc = get_program(flags, repeat=1)
    in_maps = make_in_maps(Q, K, Wq, bq, Wk, bk, Wv, bv, Wo, bo, g0, b0, g1, b1, flags)
    res = run_bass_kernel_spmd(nc, in_maps, list(range(N_CORES)))
    out = np.empty((B, NQ, D), np.float32)
    for ci in range(N_CORES):
        out[ci * BL:(ci + 1) * BL] = res.results[ci]["OUT"].reshape(BL, NQ, D)
    return out


# revision 66
# speedup vs baseline: 4052.1295x; 1.0211x over previous
"""Trainium2 Bass kernel for MAB (multihead attention block) — nn_MAB_48412871360901.

Data-parallel over batch: 16 batches -> 8 NeuronCores, 2 batches/core.
Per core, per batch (all matmuls bf16 with fp32 PSUM accumulation):
  P1  Q,K loaded natural (batched 4-chunk DMAs), cast bf16, transposed to
      QT/KT [dv, nq] layout via DMA xbar transpose (no PE involvement)
  P2  projections qT = Wq^T QT, kT = Wk^T KT (transposed layout), v = K Wv (natural)
  P3  per (head-pair, q-chunk): S^T = k^T.T q^T (row-packed 2 heads),
      exp on ACT (scale 1/sqrt(512) fused), softmax denominators Z via
      ones-matmuls, PV U^T = v^T expS^T (col-packed 2 heads),
      X^T = U^T * (1/Z)bcast + qT   (residual uses post-projection q)
  P4  LN0 in transposed layout: stats via ones-matmuls over partitions,
      rstd = exp(-0.5 ln(var+eps)) on ACT, normalize with PE-broadcast tiles
  P5  Xn natural obtained via DMA xbar transpose of XnT; M = Xn Wo (natural
      out from XnT lhsT); relu; residual; LN1 natural (bn_stats); batched
      DMA out fp32.

All scalar-engine activations are Exp/Ln, forced into the single
natural_log_exp_and_others table set to avoid ACT table-load thrash.
"""

import sys
import functools
from contextlib import ExitStack
import numpy as np
import ml_dtypes

for _p in ("/opt/trn_rl_repo", "/root/.axon_site/_ro/trn_rl_repo"):
    if _p not in sys.path:
        sys.path.insert(0, _p)

import concourse.bacc as bacc
import concourse.mybir as mybir
import concourse.tile as tile
from concourse.bass_utils import run_bass_kernel_spmd
from concourse.hw_specs import get_activation_tables as _orig_gat

BF16 = mybir.dt.bfloat16
F32 = mybir.dt.float32
NBF = ml_dtypes.bfloat16
AF = mybir.ActivationFunctionType
OP = mybir.AluOpType

B, NQ, NK = 16, 1024, 1024
D = 512
H = 8
N_CORES = 8
BL = B // N_CORES          # batches per core
EPS = 1e-5
SCALE = 1.0 / np.sqrt(512.0)

_ONE_SET = "natural_log_exp_and_others"


@functools.cache
def _gat_one_set(arch):
    """Empty out every activation-table set except the one containing both
    Exp and Ln, so bacc's table-load pass emits a single LoadActFuncSet
    instead of thrashing between exp_and_others and natural_log.
    Set indices (act_func_set_id) are preserved."""
    tabs = _orig_gat(arch)
    return {name: (fns if name == _ONE_SET else frozenset())
            for name, fns in tabs.items()}


bacc.get_activation_tables = _gat_one_set

_cache = {}


class _Ctx:
    pass


def _setup_consts(nc, cx, cst, flags):
    (bq_nz, bk_nz, bv_nz, bo_nz, ln0_aff, ln1_aff) = flags

    def din(name, shape, dt=BF16):
        return nc.dram_tensor(name, list(shape), dt, kind="ExternalInput").ap()

    def ldc(name, dshape, shape, rearr=None):
        d = din(name, dshape)
        t = cst.tile(list(shape), BF16, tag=name)
        nc.sync.dma_start(out=t, in_=d if rearr is None else d.rearrange(rearr, p=128))
        return t

    def ldf(name, shape):
        d = din(name, shape, F32)
        t = cst.tile(list(shape), F32, tag=name)
        nc.sync.dma_start(out=t, in_=d)
        return t

    cx.w_q = ldc("Wqb", (D, D), (128, 4, D), "(kt p) c -> p kt c")
    cx.w_k = ldc("Wkb", (D, D), (128, 4, D), "(kt p) c -> p kt c")
    cx.w_v = ldc("Wvb", (D, D), (128, 4, D), "(kt p) c -> p kt c")
    cx.w_o = ldc("Wob", (D, D), (128, 4, D), "(kt p) c -> p kt c")
    cx.onesc = ldc("onesc", (128, 1), (128, 1))
    cx.ones64 = ldc("ones64", (128, 64), (128, 64))
    cx.epsP = cst.tile([128, 1], F32, tag="epsP"); nc.vector.memset(cx.epsP, EPS)
    cx.eps1 = cst.tile([1, 1], F32, tag="eps1"); nc.vector.memset(cx.eps1, EPS)
    cx.bq4 = ldf("bq4", (128, 4)) if bq_nz else None
    cx.bk4 = ldf("bk4", (128, 4)) if bk_nz else None
    cx.bvb = ldf("bvb", (128, D)) if bv_nz else None
    cx.bob = ldf("bob", (128, D)) if bo_nz else None
    cx.g04 = ldf("g04", (128, 4)) if ln0_aff else None
    cx.b04 = ldf("b04", (128, 4)) if ln0_aff else None
    cx.g1b = ldf("g1b", (128, D)) if ln1_aff else None
    cx.b1b = ldf("b1b", (128, D)) if ln1_aff else None


def _p1_transpose(nc, cx, rb, src_dram, tag, eng=None):
    """DMA-xbar-transpose bf16 input (host-cast) straight from DRAM into the
    [dv, nq] layout.

    dma_start_transpose maps transposed row r of a [128, 512] input to
    out[p, kt, c] with r = kt*128 + p — the same (kt p) layout the weight
    tiles use, so the projection matmuls consume dst directly."""
    eng = eng or nc.sync
    dst = cx.p_qkt.tile([128, 4, NQ], BF16, tag=tag)
    for i in range(8):
        eng.dma_start_transpose(
            out=dst[:, :, 128 * i:128 * (i + 1)],
            in_=src_dram[rb + 128 * i: rb + 128 * (i + 1), :])
    return dst


def _p2_alloc(cx):
    qT = cx.p_proj.tile([128, 4, NQ], BF16, tag="qT")
    kT = cx.p_proj.tile([128, 4, NQ], BF16, tag="kT")
    vT = cx.p_proj.tile([128, 8, D], BF16, tag="vT")
    return qT, kT, vT


def _p2_chains(nc, cx, proj, QT, KT):
    """Return the 24 projection chains (4 PE matmuls + evac each) as
    closures, so the body can interleave their EMISSION into other phases —
    each engine's instruction stream executes in program order, so filler
    work must be woven in at emission time."""
    qT, kT, vT = proj
    chains = []

    def qk_chain(dst, w, srcT, bias, dvt, qc):
        def emit():
            pp = cx.ps_half.tile([128, D], F32, tag="h")
            for kt in range(4):
                nc.tensor.matmul(
                    pp, lhsT=w[:, kt, 128 * dvt:128 * (dvt + 1)],
                    rhs=srcT[:, kt, 512 * qc:512 * (qc + 1)],
                    start=(kt == 0), stop=(kt == 3))
            o = dst[:, dvt, 512 * qc:512 * (qc + 1)]
            if bias is not None:
                nc.vector.tensor_scalar_add(out=o, in0=pp, scalar1=bias[:, dvt:dvt + 1])
            else:
                nc.vector.tensor_copy(out=o, in_=pp)
        return emit

    def v_chain(nkt):
        def emit():
            pp = cx.ps_half.tile([128, D], F32, tag="h")
            for kt in range(4):
                nc.tensor.matmul(pp, lhsT=KT[:, kt, 128 * nkt:128 * (nkt + 1)],
                                 rhs=cx.w_v[:, kt, :], start=(kt == 0), stop=(kt == 3))
            if cx.bvb is not None:
                nc.vector.scalar_tensor_tensor(out=vT[:, nkt, :], in0=pp, scalar=0.0,
                                               in1=cx.bvb, op0=OP.add, op1=OP.add)
            else:
                nc.vector.tensor_copy(out=vT[:, nkt, :], in_=pp)
        return emit

    for dvt in range(4):
        for qc in range(2):
            chains.append(qk_chain(qT, cx.w_q, QT, cx.bq4, dvt, qc))
    for dvt in range(4):
        for qc in range(2):
            chains.append(qk_chain(kT, cx.w_k, KT, cx.bk4, dvt, qc))
    for nkt in range(8):
        chains.append(v_chain(nkt))
    return chains


def _p3_batch(nc, cx, qT, kT, vT, XT, SQ, filler=None, fill_steps=()):
    """All 8 attention units of one batch as a single software-pipelined
    stream: the score matmuls run one (unit, kt) step ahead of the
    exp->Z/PV consumers ACROSS unit boundaries, so the in-order PE stream
    never stalls on the current step's exp."""
    from collections import deque
    units = [(hp, qc) for hp in range(4) for qc in range(2)]

    def scores(hp, qc, kt):
        ps_s = cx.ps_wide.tile([128, 2 * D], F32, tag="w")
        nc.tensor.matmul(
            ps_s[:, 0:D],
            lhsT=kT[0:64, hp, 128 * kt:128 * (kt + 1)],
            rhs=qT[0:64, hp, 512 * qc:512 * (qc + 1)],
            start=True, stop=True, tile_position=(0, 0))
        nc.tensor.matmul(
            ps_s[:, D:2 * D],
            lhsT=kT[64:128, hp, 128 * kt:128 * (kt + 1)],
            rhs=qT[64:128, hp, 512 * qc:512 * (qc + 1)],
            start=True, stop=True, tile_position=(64, 0))
        return ps_s

    steps = [(u, kt) for u in range(8) for kt in range(8)]
    sq = deque()
    cur = {}
    sq.append(scores(*units[0], 0))
    for i, (u, kt) in enumerate(steps):
        hp, qc = units[u]
        if kt == 0:
            ps_u_new = cx.ps_pv.tile([128, D], F32, tag="u")
            ps_z_new = cx.ps_z.tile([128, D], F32, tag="z")
            cur[u] = (ps_u_new, ps_z_new)
        ps_u, ps_z = cur[u]
        if i + 1 < len(steps):
            un, ktn = steps[i + 1]
            sq.append(scores(*units[un], ktn))
        ps_s = sq.popleft()
        ex = cx.p_ex.tile([128, 2 * D], BF16)
        nc.scalar.activation(out=ex, in_=ps_s, func=AF.Exp, scale=SCALE)
        nc.tensor.matmul(ps_z[0:64, :], lhsT=cx.ones64, rhs=ex[:, 0:D],
                         start=(kt == 0), stop=(kt == 7), tile_position=(0, 0))
        nc.tensor.matmul(ps_z[64:128, :], lhsT=cx.ones64, rhs=ex[:, D:2 * D],
                         start=(kt == 0), stop=(kt == 7), tile_position=(0, 64))
        nc.tensor.matmul(ps_u[0:64, :],
                         lhsT=vT[:, kt, 128 * hp:128 * hp + 64],
                         rhs=ex[:, 0:D],
                         start=(kt == 0), stop=(kt == 7), tile_position=(0, 0))
        nc.tensor.matmul(ps_u[64:128, :],
                         lhsT=vT[:, kt, 128 * hp + 64:128 * (hp + 1)],
                         rhs=ex[:, D:2 * D],
                         start=(kt == 0), stop=(kt == 7), tile_position=(0, 64))
        if kt != 7 and filler and i in fill_steps:
            filler.popleft()()
        if kt == 7:
            # Unit epilogue.  ps_z rows 0:64 all hold Z(head 2hp), rows
            # 64:128 all Z(head 2hp+1) (ones64 lhsT, same N-bound matmul
            # cost as an M=1 reduction), so the reciprocal is already
            # partition-broadcast.
            rz = cx.p_rzb.tile([128, D], BF16, tag="rzb")
            with nc.allow_low_precision(reason="softmax 1/Z in bf16"):
                nc.vector.reciprocal(out=rz, in_=ps_z)
            t1 = cx.p_t1.tile([128, D], BF16, tag="t")
            nc.vector.tensor_tensor(out=t1, in0=ps_u, in1=rz, op=OP.mult)
            xs = XT[:, hp, 512 * qc:512 * (qc + 1)]
            nc.vector.tensor_tensor(out=xs, in0=t1,
                                    in1=qT[:, hp, 512 * qc:512 * (qc + 1)],
                                    op=OP.add)
            nc.vector.tensor_tensor(out=SQ[:, hp, 512 * qc:512 * (qc + 1)],
                                    in0=xs, in1=xs, op=OP.mult)
            del cur[u]
            # Last-step fillers run AFTER the epilogue so they don't delay
            # the XT/SQ writes that the next phase (LN0 stats) waits on.
            if filler and i in fill_steps:
                filler.popleft()()


def _p4_ln0_qc(nc, cx, XT, SQ, ln0_aff, XnT, qc):
    """LN0 stats + normalize for one q-half; per-qc small-tile tags so the
    two halves' scalar chains don't serialize through shared slots."""
    ps_st = cx.ps_half.tile([128, D], F32, tag="h")
    for dvt in range(4):
        nc.tensor.matmul(ps_st[0:1, :], lhsT=cx.onesc,
                         rhs=XT[:, dvt, 512 * qc:512 * (qc + 1)],
                         start=(dvt == 0), stop=(dvt == 3), tile_position=(0, 0))
        nc.tensor.matmul(ps_st[32:33, :], lhsT=cx.onesc,
                         rhs=SQ[:, dvt, 512 * qc:512 * (qc + 1)],
                         start=(dvt == 0), stop=(dvt == 3), tile_position=(0, 32))
    mu = cx.p_sml.tile([1, D], F32, tag=f"mu{qc}")
    nc.vector.tensor_scalar_mul(out=mu, in0=ps_st[0:1, :], scalar1=1.0 / D)
    mu2 = cx.p_sml.tile([1, D], BF16, tag=f"tmp{qc}")
    nc.vector.tensor_tensor(out=mu2, in0=mu, in1=mu, op=OP.mult)
    var = cx.p_sml.tile([1, D], F32, tag=f"var{qc}")
    nc.vector.scalar_tensor_tensor(out=var, in0=ps_st[32:33, :],
                                   scalar=1.0 / D, in1=mu2,
                                   op0=OP.mult, op1=OP.subtract)
    lnv = cx.p_sml.tile([1, D], BF16, tag=f"tmp{qc}")
    nc.scalar.activation(out=lnv, in_=var, func=AF.Ln, bias=cx.eps1, scale=1.0)
    rstd = cx.p_sml.tile([1, D], BF16, tag=f"rstd{qc}")
    nc.scalar.activation(out=rstd, in_=lnv, func=AF.Exp, scale=-0.5)
    nmr = cx.p_sml.tile([1, D], BF16, tag=f"nmr{qc}")
    nc.vector.scalar_tensor_tensor(out=nmr, in0=mu, scalar=-1.0, in1=rstd,
                                   op0=OP.mult, op1=OP.mult)
    rstdb = cx.p_rzb.tile([128, D], BF16, tag="rstdb")
    nc.gpsimd.partition_broadcast(rstdb, rstd)
    nmrb = cx.p_rzb.tile([128, D], BF16, tag="nmrb")
    nc.gpsimd.partition_broadcast(nmrb, nmr)
    for dvt in range(4):
        t2 = cx.p_t1.tile([128, D], BF16, tag="t")
        nc.vector.tensor_tensor(out=t2, in0=XT[:, dvt, 512 * qc:512 * (qc + 1)],
                                in1=rstdb, op=OP.mult)
        xn = XnT[:, dvt, 512 * qc:512 * (qc + 1)]
        nc.vector.tensor_tensor(out=xn, in0=t2, in1=nmrb, op=OP.add)
        if ln0_aff:
            nc.vector.tensor_scalar(out=xn, in0=xn,
                                    scalar1=cx.g04[:, dvt:dvt + 1],
                                    scalar2=cx.b04[:, dvt:dvt + 1],
                                    op0=OP.mult, op1=OP.add)


def _p4_ln0(nc, cx, XT, SQ, ln0_aff):
    XnT = cx.p_xnt.tile([128, 4, NQ], BF16)
    for qc in range(2):
        _p4_ln0_qc(nc, cx, XT, SQ, ln0_aff, XnT, qc)
    return XnT


class _P5State:
    pass


def _p5_start(nc, cx, XnT):
    # Xn natural [nq, dv] via DMA xbar transpose: out[p, nqt, dvt, c] with
    # nq = nqt*128 + p, dv = dvt*128 + c.  ACT-issued: separate HWDGE queue
    # from the P1 input transposes on SP.
    st = _P5State()
    st.XnT = XnT
    st.Xn = cx.p_xn.tile([128, 8, 4, 128], BF16, tag="Xn")
    for dvt in range(4):
        nc.scalar.dma_start_transpose(out=st.Xn[:, :, dvt, :], in_=XnT[:, dvt, :])
    st.xpre_l, st.mv_l = [], []
    vars4_0 = cx.p_sml.tile([128, 4], F32, tag="vars4_0")
    vars4_1 = cx.p_sml.tile([128, 4], F32, tag="vars4_1")
    st.vars4 = [vars4_0, vars4_1]
    return st


def _p5_chunk(nc, cx, st, nqt):
    XnT, Xn = st.XnT, st.Xn
    ps_m = cx.ps_half.tile([128, D], F32, tag="h")
    for dvt in range(4):
        nc.tensor.matmul(ps_m, lhsT=XnT[:, dvt, 128 * nqt:128 * (nqt + 1)],
                         rhs=cx.w_o[:, dvt, :],
                         start=(dvt == 0), stop=(dvt == 3))
    rl = cx.p_t1.tile([128, D], BF16, tag="t")
    if cx.bob is not None:
        tb = cx.p_t1.tile([128, D], BF16, tag="t")
        nc.vector.tensor_tensor(out=tb, in0=cx.bob, in1=ps_m, op=OP.add)
        nc.vector.tensor_scalar_max(out=rl, in0=tb, scalar1=0.0)
    else:
        nc.vector.tensor_scalar_max(out=rl, in0=ps_m, scalar1=0.0)
    xpre = cx.p_xp.tile([128, D], F32)
    nc.vector.tensor_tensor(out=xpre, in0=rl,
                            in1=Xn[:, nqt, :, :].rearrange("p a b -> p (a b)"),
                            op=OP.add)
    bst = cx.p_bst.tile([128, 6], F32, tag="bst")
    nc.vector.bn_stats(out=bst, in_=xpre)
    mv = cx.p_mv.tile([128, 2], F32, tag="mv")
    nc.vector.bn_aggr(out=mv, in_=bst)
    nc.vector.tensor_copy(out=st.vars4[nqt // 4][:, nqt % 4:nqt % 4 + 1],
                          in_=mv[:, 1:2])
    st.xpre_l.append(xpre); st.mv_l.append(mv)


def _p5_finish_g(nc, cx, st, dOut, rb, ln1_aff, g):
    """LN1 rstd + final normalize + store for one group of 4 nq-chunks, so
    group 0's output DMA can overlap group 1's chunks."""
    lnv4 = cx.p_sml.tile([128, 4], F32, tag=f"lnv4_{g}")
    nc.scalar.activation(out=lnv4, in_=st.vars4[g], func=AF.Ln, bias=cx.epsP, scale=1.0)
    rstd4 = cx.p_sml.tile([128, 4], F32, tag=f"rstd4_{g}")
    nc.scalar.activation(out=rstd4, in_=lnv4, func=AF.Exp, scale=-0.5)
    out4 = cx.p_out.tile([128, 4, D], F32)
    for c in range(4):
        nqt = 4 * g + c
        ot = out4[:, c, :]
        nc.vector.tensor_scalar(out=ot, in0=st.xpre_l[nqt],
                                scalar1=st.mv_l[nqt][:, 0:1],
                                scalar2=rstd4[:, c:c + 1],
                                op0=OP.subtract, op1=OP.mult)
        if ln1_aff:
            nc.vector.tensor_tensor(out=ot, in0=ot, in1=cx.g1b, op=OP.mult)
            nc.vector.tensor_tensor(out=ot, in0=ot, in1=cx.b1b, op=OP.add)
    nc.scalar.dma_start(
        out=dOut[rb + 512 * g: rb + 512 * (g + 1), :].rearrange(
            "(c p) d -> p c d", p=128),
        in_=out4)


def _build(flags, repeat=1):
    (bq_nz, bk_nz, bv_nz, bo_nz, ln0_aff, ln1_aff) = flags
    nc = bacc.Bacc("TRN2", target_bir_lowering=False, debug=False,
                   num_devices=N_CORES)

    dQ = nc.dram_tensor("Qs", [BL * NQ, D], BF16, kind="ExternalInput").ap()
    dK = nc.dram_tensor("Ks", [BL * NK, D], BF16, kind="ExternalInput").ap()
    dOut = nc.dram_tensor("OUT", [BL * NQ, D], F32, kind="ExternalOutput").ap()

    cx = _Ctx()
    with ExitStack() as es:
        tc = es.enter_context(tile.TileContext(nc))
        ec = es.enter_context
        cst = ec(tc.tile_pool(name="cst", bufs=1))
        cx.p_qkt = ec(tc.tile_pool(name="qkt", bufs=1))
        cx.p_proj = ec(tc.tile_pool(name="proj", bufs=2))
        cx.p_xt = ec(tc.tile_pool(name="xt", bufs=2))
        cx.p_xnt = ec(tc.tile_pool(name="xnt", bufs=2))
        cx.p_xn = ec(tc.tile_pool(name="xn", bufs=1))
        cx.p_ex = ec(tc.tile_pool(name="ex", bufs=3))
        cx.p_rzb = ec(tc.tile_pool(name="rzb", bufs=2))
        cx.p_t1 = ec(tc.tile_pool(name="t1", bufs=3))
        cx.p_xp = ec(tc.tile_pool(name="xp", bufs=9))
        cx.p_out = ec(tc.tile_pool(name="outp", bufs=1))
        cx.p_sml = ec(tc.tile_pool(name="sml", bufs=1))
        cx.p_mv = ec(tc.tile_pool(name="mv", bufs=10))
        cx.p_bst = ec(tc.tile_pool(name="bst", bufs=2))
        cx.ps_wide = ec(tc.tile_pool(name="wide", bufs=2, space="PSUM"))
        cx.ps_half = ec(tc.tile_pool(name="half", bufs=2, space="PSUM"))
        cx.ps_pv = ec(tc.tile_pool(name="pv", bufs=1, space="PSUM"))
        cx.ps_z = ec(tc.tile_pool(name="z", bufs=1, space="PSUM"))
        _setup_consts(nc, cx, cst, flags)

        def body():
            from collections import deque
            units = [(hp, qc) for hp in range(4) for qc in range(2)]
            # Batch 0 inputs, then batch 1 inputs (all DMA-only, queue early)
            QT0 = _p1_transpose(nc, cx, 0, dQ, "QT")
            KT0 = _p1_transpose(nc, cx, 0, dK, "KT")
            proj0 = _p2_alloc(cx)
            for c in _p2_chains(nc, cx, proj0, QT0, KT0):
                c()
            QT1 = _p1_transpose(nc, cx, NQ, dQ, "QT")
            KT1 = _p1_transpose(nc, cx, NQ, dK, "KT")
            qT0, kT0, vT0 = proj0
            # P3 batch 0, with batch-1 projection chains woven in as PE
            # filler (P3 is ACT-bound; each engine stream runs in program
            # order, so filler must be emitted inline).
            proj1 = _p2_alloc(cx)
            XT0 = cx.p_xt.tile([128, 4, NQ], BF16, tag="XT")
            SQ0 = cx.p_xt.tile([128, 4, NQ], BF16, tag="SQ")
            XnT0 = cx.p_xnt.tile([128, 4, NQ], BF16)

            def f_p4_qc0():
                # qc0 columns of XT0/SQ0 are complete after unit 6 (3,0),
                # so this can run during the last (qc=1) unit of P3_b0.
                _p4_ln0_qc(nc, cx, XT0, SQ0, ln0_aff, XnT0, 0)

            # fills at steps 8u+{2,4,6} (24 P2_b1 chains) + step 63 (P4 qc0)
            fill = deque(_p2_chains(nc, cx, proj1, QT1, KT1) + [f_p4_qc0])
            fsteps = {8 * u + k for u in range(8) for k in (2, 4, 6)} | {63}
            _p3_batch(nc, cx, qT0, kT0, vT0, XT0, SQ0, filler=fill,
                      fill_steps=fsteps)
            qT1, kT1, vT1 = proj1
            # P3 batch 1, with batch-0 epilogue (P4 qc1 + P5) woven in.
            XT1 = cx.p_xt.tile([128, 4, NQ], BF16, tag="XT")
            SQ1 = cx.p_xt.tile([128, 4, NQ], BF16, tag="SQ")
            XnT1 = cx.p_xnt.tile([128, 4, NQ], BF16)
            st0_box = []

            def f_p4_qc1():
                _p4_ln0_qc(nc, cx, XT0, SQ0, ln0_aff, XnT0, 1)
                st0_box.append(_p5_start(nc, cx, XnT0))

            def f_chunk(nqt):
                return lambda: _p5_chunk(nc, cx, st0_box[0], nqt)

            def f_finish(g):
                return lambda: _p5_finish_g(nc, cx, st0_box[0], dOut, 0, ln1_aff, g)

            def f_p4b1_qc0():
                _p4_ln0_qc(nc, cx, XT1, SQ1, ln0_aff, XnT1, 0)

            fill = deque([f_p4_qc1,
                          f_chunk(0), f_chunk(1), f_chunk(2), f_chunk(3),
                          f_finish(0),
                          f_chunk(4), f_chunk(5), f_chunk(6), f_chunk(7),
                          f_finish(1),
                          f_p4b1_qc0])
            fsteps = ({2} | {8 * u + k for u in range(1, 6) for k in (2, 5)}
                      | {58})
            _p3_batch(nc, cx, qT1, kT1, vT1, XT1, SQ1, filler=fill,
                      fill_steps=fsteps)
            while fill:
                fill.popleft()()
            # Batch 1 epilogue (tail)
            _p4_ln0_qc(nc, cx, XT1, SQ1, ln0_aff, XnT1, 1)
            st1 = _p5_start(nc, cx, XnT1)
            for nqt in range(4):
                _p5_chunk(nc, cx, st1, nqt)
            _p5_finish_g(nc, cx, st1, dOut, NQ, ln1_aff, 0)
            for nqt in range(4, 8):
                _p5_chunk(nc, cx, st1, nqt)
            _p5_finish_g(nc, cx, st1, dOut, NQ, ln1_aff, 1)

        if repeat == 1:
            body()
        else:
            # Branch hints: the body far exceeds one IRAM block per engine,
            # so the back-edge would I$-miss (~4us) without prefetch hints.
            hints = (mybir.EngineType.PE, mybir.EngineType.DVE,
                     mybir.EngineType.Activation, mybir.EngineType.Pool,
                     mybir.EngineType.SP)
            with tc.For_i(0, repeat, 1, hint_engines=hints):
                body()

    nc.compile()
    return nc


def _consts(Wq, Wk, Wv, Wo, flags, bq, bk, bv, bo, g0, b0, g1, b1):
    (bq_nz, bk_nz, bv_nz, bo_nz, ln0_aff, ln1_aff) = flags
    c = {
        "Wqb": np.ascontiguousarray(np.asarray(Wq).astype(NBF)),
        "Wkb": np.ascontiguousarray(np.asarray(Wk).astype(NBF)),
        "Wvb": np.ascontiguousarray(np.asarray(Wv).astype(NBF)),
        "Wob": np.ascontiguousarray(np.asarray(Wo).astype(NBF)),
        "onesc": np.ones((128, 1), NBF),
        "ones64": np.ones((128, 64), NBF),
    }
    if bq_nz: c["bq4"] = np.ascontiguousarray(np.asarray(bq).reshape(4, 128).T.astype(np.float32))
    if bk_nz: c["bk4"] = np.ascontiguousarray(np.asarray(bk).reshape(4, 128).T.astype(np.float32))
    if bv_nz: c["bvb"] = np.ascontiguousarray(np.broadcast_to(np.asarray(bv, np.float32), (128, D)))
    if bo_nz: c["bob"] = np.ascontiguousarray(np.broadcast_to(np.asarray(bo, np.float32), (128, D)))
    if ln0_aff:
        c["g04"] = np.ascontiguousarray(np.asarray(g0).reshape(4, 128).T.astype(np.float32))
        c["b04"] = np.ascontiguousarray(np.asarray(b0).reshape(4, 128).T.astype(np.float32))
    if ln1_aff:
        c["g1b"] = np.ascontiguousarray(np.broadcast_to(np.asarray(g1, np.float32), (128, D)))
        c["b1b"] = np.ascontiguousarray(np.broadcast_to(np.asarray(b1, np.float32), (128, D)))
    return c


def make_in_maps(Q, K, Wq, bq, Wk, bk, Wv, bv, Wo, bo, g0, b0, g1, b1, flags):
    consts = _consts(Wq, Wk, Wv, Wo, flags, bq, bk, bv, bo, g0, b0, g1, b1)
    in_maps = []
    for ci in range(N_CORES):
        m = dict(consts)
        m["Qs"] = np.ascontiguousarray(
            np.asarray(Q)[ci * BL:(ci + 1) * BL].reshape(BL * NQ, D).astype(NBF))
        m["Ks"] = np.ascontiguousarray(
            np.asarray(K)[ci * BL:(ci + 1) * BL].reshape(BL * NK, D).astype(NBF))
        in_maps.append(m)
    return in_maps


def get_flags(bq, bk, bv, bo, g0, b0, g1, b1):
    return (bool(np.any(np.asarray(bq))), bool(np.any(np.asarray(bk))),
            bool(np.any(np.asarray(bv))), bool(np.any(np.asarray(bo))),
            bool(np.any(np.asarray(g0) != 1) or np.any(np.asarray(b0))),
            bool(np.any(np.asarray(g1) != 1) or np.any(np.asarray(b1))))


def get_program(flags, repeat=1):
    key = (flags, repeat)
    if key not in _cache:
        _cache[key] = _build(flags, repeat)
    return _cache[key]


def kernel(Q, K, Wq, bq, Wk, bk, Wv, bv, Wo, bo, g0, b0, g1, b1):
    flags = get_flags(bq, bk, bv, bo, g0, b0, g1, b1)
    nc = get_program(flags, repeat=1)
    in_maps = make_in_maps(Q, K, Wq, bq, Wk, bk, Wv, bv, Wo, bo, g0, b0, g1, b1, flags)
    res = run_bass_kernel_spmd(nc, in_maps, list(range(N_CORES)))
    out = np.empty((B, NQ, D), np.float32)
    for ci in range(N_CORES):
        out[ci * BL:(ci + 1) * BL] = res.results[ci]["OUT"].reshape(BL, NQ, D)
    return out


# revision 69
# speedup vs baseline: 4113.5779x; 1.0152x over previous
"""Trainium2 Bass kernel for MAB (multihead attention block) — nn_MAB_48412871360901.

Data-parallel over batch: 16 batches -> 8 NeuronCores, 2 batches/core.
Per core, per batch (all matmuls bf16 with fp32 PSUM accumulation):
  P1  Q,K loaded natural (batched 4-chunk DMAs), cast bf16, transposed to
      QT/KT [dv, nq] layout via DMA xbar transpose (no PE involvement)
  P2  projections qT = Wq^T QT, kT = Wk^T KT (transposed layout), v = K Wv (natural)
  P3  per (head-pair, q-chunk): S^T = k^T.T q^T (row-packed 2 heads),
      exp on ACT (scale 1/sqrt(512) fused), softmax denominators Z via
      ones-matmuls, PV U^T = v^T expS^T (col-packed 2 heads),
      X^T = U^T * (1/Z)bcast + qT   (residual uses post-projection q)
  P4  LN0 in transposed layout: stats via ones-matmuls over partitions,
      rstd = exp(-0.5 ln(var+eps)) on ACT, normalize with PE-broadcast tiles
  P5  Xn natural obtained via DMA xbar transpose of XnT; M = Xn Wo (natural
      out from XnT lhsT); relu; residual; LN1 natural (bn_stats); batched
      DMA out fp32.

All scalar-engine activations are Exp/Ln, forced into the single
natural_log_exp_and_others table set to avoid ACT table-load thrash.
"""

import sys
import functools
from contextlib import ExitStack
import numpy as np
import ml_dtypes

for _p in ("/opt/trn_rl_repo", "/root/.axon_site/_ro/trn_rl_repo"):
    if _p not in sys.path:
        sys.path.insert(0, _p)

import concourse.bacc as bacc
import concourse.mybir as mybir
import concourse.tile as tile
from concourse.bass_utils import run_bass_kernel_spmd
from concourse.hw_specs import get_activation_tables as _orig_gat

BF16 = mybir.dt.bfloat16
F32 = mybir.dt.float32
NBF = ml_dtypes.bfloat16
AF = mybir.ActivationFunctionType
OP = mybir.AluOpType

B, NQ, NK = 16, 1024, 1024
D = 512
H = 8
N_CORES = 8
BL = B // N_CORES          # batches per core
EPS = 1e-5
SCALE = 1.0 / np.sqrt(512.0)

_ONE_SET = "natural_log_exp_and_others"


@functools.cache
def _gat_one_set(arch):
    """Empty out every activation-table set except the one containing both
    Exp and Ln, so bacc's table-load pass emits a single LoadActFuncSet
    instead of thrashing between exp_and_others and natural_log.
    Set indices (act_func_set_id) are preserved."""
    tabs = _orig_gat(arch)
    return {name: (fns if name == _ONE_SET else frozenset())
            for name, fns in tabs.items()}


bacc.get_activation_tables = _gat_one_set

_cache = {}


class _Ctx:
    pass


def _setup_consts(nc, cx, cst, flags):
    (bq_nz, bk_nz, bv_nz, bo_nz, ln0_aff, ln1_aff) = flags

    def din(name, shape, dt=BF16):
        return nc.dram_tensor(name, list(shape), dt, kind="ExternalInput").ap()

    def ldc(name, dshape, shape, rearr=None):
        d = din(name, dshape)
        t = cst.tile(list(shape), BF16, tag=name)
        nc.sync.dma_start(out=t, in_=d if rearr is None else d.rearrange(rearr, p=128))
        return t

    def ldf(name, shape):
        d = din(name, shape, F32)
        t = cst.tile(list(shape), F32, tag=name)
        nc.sync.dma_start(out=t, in_=d)
        return t

    cx.w_q = ldc("Wqb", (D, D), (128, 4, D), "(kt p) c -> p kt c")
    cx.w_k = ldc("Wkb", (D, D), (128, 4, D), "(kt p) c -> p kt c")
    cx.w_v = ldc("Wvb", (D, D), (128, 4, D), "(kt p) c -> p kt c")
    cx.w_o = ldc("Wob", (D, D), (128, 4, D), "(kt p) c -> p kt c")
    cx.onesc = ldc("onesc", (128, 1), (128, 1))
    cx.ones64 = ldc("ones64", (128, 64), (128, 64))
    cx.epsP = cst.tile([128, 1], F32, tag="epsP"); nc.vector.memset(cx.epsP, EPS)
    cx.eps1 = cst.tile([1, 1], F32, tag="eps1"); nc.vector.memset(cx.eps1, EPS)
    cx.bq4 = ldf("bq4", (128, 4)) if bq_nz else None
    cx.bk4 = ldf("bk4", (128, 4)) if bk_nz else None
    cx.bvb = ldf("bvb", (128, D)) if bv_nz else None
    cx.bob = ldf("bob", (128, D)) if bo_nz else None
    cx.g04 = ldf("g04", (128, 4)) if ln0_aff else None
    cx.b04 = ldf("b04", (128, 4)) if ln0_aff else None
    cx.g1b = ldf("g1b", (128, D)) if ln1_aff else None
    cx.b1b = ldf("b1b", (128, D)) if ln1_aff else None


def _p1_transpose(nc, cx, rb, src_dram, tag, eng=None):
    """DMA-xbar-transpose bf16 input (host-cast) straight from DRAM into the
    [dv, nq] layout.

    dma_start_transpose maps transposed row r of a [128, 512] input to
    out[p, kt, c] with r = kt*128 + p — the same (kt p) layout the weight
    tiles use, so the projection matmuls consume dst directly."""
    eng = eng or nc.sync
    dst = cx.p_qkt.tile([128, 4, NQ], BF16, tag=tag)
    for i in range(8):
        eng.dma_start_transpose(
            out=dst[:, :, 128 * i:128 * (i + 1)],
            in_=src_dram[rb + 128 * i: rb + 128 * (i + 1), :])
    return dst


def _p2_alloc(cx):
    qT = cx.p_proj.tile([128, 4, NQ], BF16, tag="qT")
    kT = cx.p_proj.tile([128, 4, NQ], BF16, tag="kT")
    vT = cx.p_proj.tile([128, 8, D], BF16, tag="vT")
    return qT, kT, vT


def _p2_chains(nc, cx, proj, QT, KT):
    """Return the 24 projection chains (4 PE matmuls + evac each) as
    closures, so the body can interleave their EMISSION into other phases —
    each engine's instruction stream executes in program order, so filler
    work must be woven in at emission time."""
    qT, kT, vT = proj
    chains = []

    def qk_chain(dst, w, srcT, bias, dvt, qc):
        def emit():
            pp = cx.ps_half.tile([128, D], F32, tag="h")
            for kt in range(4):
                nc.tensor.matmul(
                    pp, lhsT=w[:, kt, 128 * dvt:128 * (dvt + 1)],
                    rhs=srcT[:, kt, 512 * qc:512 * (qc + 1)],
                    start=(kt == 0), stop=(kt == 3))
            o = dst[:, dvt, 512 * qc:512 * (qc + 1)]
            if bias is not None:
                nc.vector.tensor_scalar_add(out=o, in0=pp, scalar1=bias[:, dvt:dvt + 1])
            else:
                nc.vector.tensor_copy(out=o, in_=pp)
        return emit

    def v_chain(nkt):
        def emit():
            pp = cx.ps_half.tile([128, D], F32, tag="h")
            for kt in range(4):
                nc.tensor.matmul(pp, lhsT=KT[:, kt, 128 * nkt:128 * (nkt + 1)],
                                 rhs=cx.w_v[:, kt, :], start=(kt == 0), stop=(kt == 3))
            if cx.bvb is not None:
                nc.vector.scalar_tensor_tensor(out=vT[:, nkt, :], in0=pp, scalar=0.0,
                                               in1=cx.bvb, op0=OP.add, op1=OP.add)
            else:
                nc.vector.tensor_copy(out=vT[:, nkt, :], in_=pp)
        return emit

    for dvt in range(4):
        for qc in range(2):
            chains.append(qk_chain(qT, cx.w_q, QT, cx.bq4, dvt, qc))
    for dvt in range(4):
        for qc in range(2):
            chains.append(qk_chain(kT, cx.w_k, KT, cx.bk4, dvt, qc))
    for nkt in range(8):
        chains.append(v_chain(nkt))
    return chains


def _p3_batch(nc, cx, qT, kT, vT, XT, SQ, filler=None, fill_steps=()):
    """All 8 attention units of one batch as a single software-pipelined
    stream: the score matmuls run one (unit, kt) step ahead of the
    exp->Z/PV consumers ACROSS unit boundaries, so the in-order PE stream
    never stalls on the current step's exp."""
    from collections import deque
    units = [(hp, qc) for hp in range(4) for qc in range(2)]

    def scores(hp, qc, kt):
        ps_s = cx.ps_wide.tile([128, 2 * D], F32, tag="w")
        nc.tensor.matmul(
            ps_s[:, 0:D],
            lhsT=kT[0:64, hp, 128 * kt:128 * (kt + 1)],
            rhs=qT[0:64, hp, 512 * qc:512 * (qc + 1)],
            start=True, stop=True, tile_position=(0, 0))
        nc.tensor.matmul(
            ps_s[:, D:2 * D],
            lhsT=kT[64:128, hp, 128 * kt:128 * (kt + 1)],
            rhs=qT[64:128, hp, 512 * qc:512 * (qc + 1)],
            start=True, stop=True, tile_position=(64, 0))
        return ps_s

    steps = [(u, kt) for u in range(8) for kt in range(8)]
    sq = deque()
    cur = {}
    sq.append(scores(*units[0], 0))
    for i, (u, kt) in enumerate(steps):
        hp, qc = units[u]
        if kt == 0:
            ps_u_new = cx.ps_pv.tile([128, D], F32, tag="u")
            ps_z_new = cx.ps_z.tile([128, D], F32, tag="z")
            cur[u] = (ps_u_new, ps_z_new)
        ps_u, ps_z = cur[u]
        if i + 1 < len(steps):
            un, ktn = steps[i + 1]
            sq.append(scores(*units[un], ktn))
        ps_s = sq.popleft()
        ex = cx.p_ex.tile([128, 2 * D], BF16)
        nc.scalar.activation(out=ex, in_=ps_s, func=AF.Exp, scale=SCALE)
        nc.tensor.matmul(ps_z[0:64, :], lhsT=cx.ones64, rhs=ex[:, 0:D],
                         start=(kt == 0), stop=(kt == 7), tile_position=(0, 0))
        nc.tensor.matmul(ps_z[64:128, :], lhsT=cx.ones64, rhs=ex[:, D:2 * D],
                         start=(kt == 0), stop=(kt == 7), tile_position=(0, 64))
        nc.tensor.matmul(ps_u[0:64, :],
                         lhsT=vT[:, kt, 128 * hp:128 * hp + 64],
                         rhs=ex[:, 0:D],
                         start=(kt == 0), stop=(kt == 7), tile_position=(0, 0))
        nc.tensor.matmul(ps_u[64:128, :],
                         lhsT=vT[:, kt, 128 * hp + 64:128 * (hp + 1)],
                         rhs=ex[:, D:2 * D],
                         start=(kt == 0), stop=(kt == 7), tile_position=(0, 64))
        if kt != 7 and filler and i in fill_steps:
            filler.popleft()()
        if kt == 7:
            # Unit epilogue.  ps_z rows 0:64 all hold Z(head 2hp), rows
            # 64:128 all Z(head 2hp+1) (ones64 lhsT, same N-bound matmul
            # cost as an M=1 reduction), so the reciprocal is already
            # partition-broadcast.
            rz = cx.p_rzb.tile([128, D], BF16, tag="rzb")
            with nc.allow_low_precision(reason="softmax 1/Z in bf16"):
                nc.vector.reciprocal(out=rz, in_=ps_z)
            t1 = cx.p_t1.tile([128, D], BF16, tag="t")
            nc.vector.tensor_tensor(out=t1, in0=ps_u, in1=rz, op=OP.mult)
            xs = XT[:, hp, 512 * qc:512 * (qc + 1)]
            nc.vector.tensor_tensor(out=xs, in0=t1,
                                    in1=qT[:, hp, 512 * qc:512 * (qc + 1)],
                                    op=OP.add)
            nc.vector.tensor_tensor(out=SQ[:, hp, 512 * qc:512 * (qc + 1)],
                                    in0=xs, in1=xs, op=OP.mult)
            del cur[u]
            # Last-step fillers run AFTER the epilogue so they don't delay
            # the XT/SQ writes that the next phase (LN0 stats) waits on.
            if filler and i in fill_steps:
                filler.popleft()()


def _p4_stats(nc, cx, XT, SQ, qc):
    """LN0 stats + rstd scalar chain for one q-half (no normalize)."""
    ps_st = cx.ps_half.tile([128, D], F32, tag="h")
    for dvt in range(4):
        nc.tensor.matmul(ps_st[0:1, :], lhsT=cx.onesc,
                         rhs=XT[:, dvt, 512 * qc:512 * (qc + 1)],
                         start=(dvt == 0), stop=(dvt == 3), tile_position=(0, 0))
        nc.tensor.matmul(ps_st[32:33, :], lhsT=cx.onesc,
                         rhs=SQ[:, dvt, 512 * qc:512 * (qc + 1)],
                         start=(dvt == 0), stop=(dvt == 3), tile_position=(0, 32))
    mu = cx.p_sml.tile([1, D], F32, tag=f"mu{qc}")
    nc.vector.tensor_scalar_mul(out=mu, in0=ps_st[0:1, :], scalar1=1.0 / D)
    mu2 = cx.p_sml.tile([1, D], BF16, tag=f"tmp{qc}")
    nc.vector.tensor_tensor(out=mu2, in0=mu, in1=mu, op=OP.mult)
    var = cx.p_sml.tile([1, D], F32, tag=f"var{qc}")
    nc.vector.scalar_tensor_tensor(out=var, in0=ps_st[32:33, :],
                                   scalar=1.0 / D, in1=mu2,
                                   op0=OP.mult, op1=OP.subtract)
    lnv = cx.p_sml.tile([1, D], BF16, tag=f"tmp{qc}")
    nc.scalar.activation(out=lnv, in_=var, func=AF.Ln, bias=cx.eps1, scale=1.0)
    rstd = cx.p_sml.tile([1, D], BF16, tag=f"rstd{qc}")
    nc.scalar.activation(out=rstd, in_=lnv, func=AF.Exp, scale=-0.5)
    return mu, rstd


def _p4_norm(nc, cx, XT, ln0_aff, XnT, qc, mu, rstd):
    """LN0 normalize for one q-half from precomputed mu/rstd rows."""
    nmr = cx.p_sml.tile([1, D], BF16, tag=f"nmr{qc}")
    nc.vector.scalar_tensor_tensor(out=nmr, in0=mu, scalar=-1.0, in1=rstd,
                                   op0=OP.mult, op1=OP.mult)
    rstdb = cx.p_rzb.tile([128, D], BF16, tag="rstdb")
    nc.gpsimd.partition_broadcast(rstdb, rstd)
    nmrb = cx.p_rzb.tile([128, D], BF16, tag="nmrb")
    nc.gpsimd.partition_broadcast(nmrb, nmr)
    for dvt in range(4):
        t2 = cx.p_t1.tile([128, D], BF16, tag="t")
        nc.vector.tensor_tensor(out=t2, in0=XT[:, dvt, 512 * qc:512 * (qc + 1)],
                                in1=rstdb, op=OP.mult)
        xn = XnT[:, dvt, 512 * qc:512 * (qc + 1)]
        nc.vector.tensor_tensor(out=xn, in0=t2, in1=nmrb, op=OP.add)
        if ln0_aff:
            nc.vector.tensor_scalar(out=xn, in0=xn,
                                    scalar1=cx.g04[:, dvt:dvt + 1],
                                    scalar2=cx.b04[:, dvt:dvt + 1],
                                    op0=OP.mult, op1=OP.add)


def _p4_ln0_qc(nc, cx, XT, SQ, ln0_aff, XnT, qc):
    mu, rstd = _p4_stats(nc, cx, XT, SQ, qc)
    _p4_norm(nc, cx, XT, ln0_aff, XnT, qc, mu, rstd)


class _P5State:
    pass


def _p5_alloc(nc, cx, XnT):
    st = _P5State()
    st.XnT = XnT
    st.Xn = cx.p_xn.tile([128, 8, 4, 128], BF16, tag="Xn")
    st.xpre_l, st.mv_l = [], []
    vars4_0 = cx.p_sml.tile([128, 4], F32, tag="vars4_0")
    vars4_1 = cx.p_sml.tile([128, 4], F32, tag="vars4_1")
    st.vars4 = [vars4_0, vars4_1]
    return st


def _p5_xbar(nc, cx, st, qc):
    # Xn natural [nq, dv] via DMA xbar transpose for one q-half:
    # out[p, 4qc+kt', dvt, c] with nq = (4qc+kt')*128 + p, dv = dvt*128 + c.
    # ACT-issued: separate HWDGE queue from the P1 input transposes on SP.
    for dvt in range(4):
        nc.scalar.dma_start_transpose(
            out=st.Xn[:, 4 * qc:4 * (qc + 1), dvt, :],
            in_=st.XnT[:, dvt, 512 * qc:512 * (qc + 1)])


def _p5_start(nc, cx, XnT):
    st = _p5_alloc(nc, cx, XnT)
    _p5_xbar(nc, cx, st, 0)
    _p5_xbar(nc, cx, st, 1)
    return st


def _p5_chunk(nc, cx, st, nqt, relu_act=False):
    XnT, Xn = st.XnT, st.Xn
    ps_m = cx.ps_half.tile([128, D], F32, tag="h")
    for dvt in range(4):
        nc.tensor.matmul(ps_m, lhsT=XnT[:, dvt, 128 * nqt:128 * (nqt + 1)],
                         rhs=cx.w_o[:, dvt, :],
                         start=(dvt == 0), stop=(dvt == 3))
    rl = cx.p_t1.tile([128, D], BF16, tag="t")
    if cx.bob is not None:
        tb = cx.p_t1.tile([128, D], BF16, tag="t")
        nc.vector.tensor_tensor(out=tb, in0=cx.bob, in1=ps_m, op=OP.add)
        nc.vector.tensor_scalar_max(out=rl, in0=tb, scalar1=0.0)
    elif relu_act:
        # Tail chunks: ACT is idle there, DVE is the tail bottleneck.
        nc.scalar.activation(out=rl, in_=ps_m, func=AF.Relu, scale=1.0)
    else:
        nc.vector.tensor_scalar_max(out=rl, in0=ps_m, scalar1=0.0)
    xpre = cx.p_xp.tile([128, D], F32)
    nc.vector.tensor_tensor(out=xpre, in0=rl,
                            in1=Xn[:, nqt, :, :].rearrange("p a b -> p (a b)"),
                            op=OP.add)
    bst = cx.p_bst.tile([128, 6], F32, tag="bst")
    nc.vector.bn_stats(out=bst, in_=xpre)
    mv = cx.p_mv.tile([128, 2], F32, tag="mv")
    nc.vector.bn_aggr(out=mv, in_=bst)
    nc.vector.tensor_copy(out=st.vars4[nqt // 4][:, nqt % 4:nqt % 4 + 1],
                          in_=mv[:, 1:2])
    st.xpre_l.append(xpre); st.mv_l.append(mv)


def _p5_finish_g(nc, cx, st, dOut, rb, ln1_aff, g):
    """LN1 rstd + final normalize + store for one group of 4 nq-chunks, so
    group 0's output DMA can overlap group 1's chunks."""
    lnv4 = cx.p_sml.tile([128, 4], F32, tag=f"lnv4_{g}")
    nc.scalar.activation(out=lnv4, in_=st.vars4[g], func=AF.Ln, bias=cx.epsP, scale=1.0)
    rstd4 = cx.p_sml.tile([128, 4], F32, tag=f"rstd4_{g}")
    nc.scalar.activation(out=rstd4, in_=lnv4, func=AF.Exp, scale=-0.5)
    out4 = cx.p_out.tile([128, 4, D], F32)
    for c in range(4):
        nqt = 4 * g + c
        ot = out4[:, c, :]
        nc.vector.tensor_scalar(out=ot, in0=st.xpre_l[nqt],
                                scalar1=st.mv_l[nqt][:, 0:1],
                                scalar2=rstd4[:, c:c + 1],
                                op0=OP.subtract, op1=OP.mult)
        if ln1_aff:
            nc.vector.tensor_tensor(out=ot, in0=ot, in1=cx.g1b, op=OP.mult)
            nc.vector.tensor_tensor(out=ot, in0=ot, in1=cx.b1b, op=OP.add)
    nc.scalar.dma_start(
        out=dOut[rb + 512 * g: rb + 512 * (g + 1), :].rearrange(
            "(c p) d -> p c d", p=128),
        in_=out4)


def _build(flags, repeat=1):
    (bq_nz, bk_nz, bv_nz, bo_nz, ln0_aff, ln1_aff) = flags
    nc = bacc.Bacc("TRN2", target_bir_lowering=False, debug=False,
                   num_devices=N_CORES)

    dQ = nc.dram_tensor("Qs", [BL * NQ, D], BF16, kind="ExternalInput").ap()
    dK = nc.dram_tensor("Ks", [BL * NK, D], BF16, kind="ExternalInput").ap()
    dOut = nc.dram_tensor("OUT", [BL * NQ, D], F32, kind="ExternalOutput").ap()

    cx = _Ctx()
    with ExitStack() as es:
        tc = es.enter_context(tile.TileContext(nc))
        ec = es.enter_context
        cst = ec(tc.tile_pool(name="cst", bufs=1))
        cx.p_qkt = ec(tc.tile_pool(name="qkt", bufs=1))
        cx.p_proj = ec(tc.tile_pool(name="proj", bufs=2))
        cx.p_xt = ec(tc.tile_pool(name="xt", bufs=2))
        cx.p_xnt = ec(tc.tile_pool(name="xnt", bufs=2))
        cx.p_xn = ec(tc.tile_pool(name="xn", bufs=1))
        cx.p_ex = ec(tc.tile_pool(name="ex", bufs=3))
        cx.p_rzb = ec(tc.tile_pool(name="rzb", bufs=2))
        cx.p_t1 = ec(tc.tile_pool(name="t1", bufs=3))
        cx.p_xp = ec(tc.tile_pool(name="xp", bufs=9))
        cx.p_out = ec(tc.tile_pool(name="outp", bufs=1))
        cx.p_sml = ec(tc.tile_pool(name="sml", bufs=1))
        cx.p_mv = ec(tc.tile_pool(name="mv", bufs=10))
        cx.p_bst = ec(tc.tile_pool(name="bst", bufs=2))
        cx.ps_wide = ec(tc.tile_pool(name="wide", bufs=2, space="PSUM"))
        cx.ps_half = ec(tc.tile_pool(name="half", bufs=2, space="PSUM"))
        cx.ps_pv = ec(tc.tile_pool(name="pv", bufs=1, space="PSUM"))
        cx.ps_z = ec(tc.tile_pool(name="z", bufs=1, space="PSUM"))
        _setup_consts(nc, cx, cst, flags)

        def body():
            from collections import deque
            units = [(hp, qc) for hp in range(4) for qc in range(2)]
            # Batch 0 inputs, then batch 1 inputs (all DMA-only, queue early)
            QT0 = _p1_transpose(nc, cx, 0, dQ, "QT")
            KT0 = _p1_transpose(nc, cx, 0, dK, "KT")
            proj0 = _p2_alloc(cx)
            for c in _p2_chains(nc, cx, proj0, QT0, KT0):
                c()
            QT1 = _p1_transpose(nc, cx, NQ, dQ, "QT")
            KT1 = _p1_transpose(nc, cx, NQ, dK, "KT")
            qT0, kT0, vT0 = proj0
            # P3 batch 0, with batch-1 projection chains woven in as PE
            # filler (P3 is ACT-bound; each engine stream runs in program
            # order, so filler must be emitted inline).
            proj1 = _p2_alloc(cx)
            XT0 = cx.p_xt.tile([128, 4, NQ], BF16, tag="XT")
            SQ0 = cx.p_xt.tile([128, 4, NQ], BF16, tag="SQ")
            XnT0 = cx.p_xnt.tile([128, 4, NQ], BF16)

            def f_p4_qc0():
                # qc0 columns of XT0/SQ0 are complete after unit 6 (3,0),
                # so this can run during the last (qc=1) unit of P3_b0.
                _p4_ln0_qc(nc, cx, XT0, SQ0, ln0_aff, XnT0, 0)

            # fills at steps 8u+{2,4,6} (24 P2_b1 chains) + step 63 (P4 qc0)
            fill = deque(_p2_chains(nc, cx, proj1, QT1, KT1) + [f_p4_qc0])
            fsteps = {8 * u + k for u in range(8) for k in (2, 4, 6)} | {63}
            _p3_batch(nc, cx, qT0, kT0, vT0, XT0, SQ0, filler=fill,
                      fill_steps=fsteps)
            qT1, kT1, vT1 = proj1
            # P3 batch 1, with batch-0 epilogue (P4 qc1 + P5) woven in.
            XT1 = cx.p_xt.tile([128, 4, NQ], BF16, tag="XT")
            SQ1 = cx.p_xt.tile([128, 4, NQ], BF16, tag="SQ")
            XnT1 = cx.p_xnt.tile([128, 4, NQ], BF16)
            st0_box = []

            def f_p4_qc1():
                _p4_ln0_qc(nc, cx, XT0, SQ0, ln0_aff, XnT0, 1)
                st0_box.append(_p5_start(nc, cx, XnT0))

            def f_chunk(nqt):
                return lambda: _p5_chunk(nc, cx, st0_box[0], nqt)

            def f_finish(g):
                return lambda: _p5_finish_g(nc, cx, st0_box[0], dOut, 0, ln1_aff, g)

            def f_p4b1_qc0():
                _p4_ln0_qc(nc, cx, XT1, SQ1, ln0_aff, XnT1, 0)

            fill = deque([f_p4_qc1,
                          f_chunk(0), f_chunk(1), f_chunk(2), f_chunk(3),
                          f_finish(0),
                          f_chunk(4), f_chunk(5), f_chunk(6), f_chunk(7),
                          f_finish(1),
                          f_p4b1_qc0])
            fsteps = ({2} | {8 * u + k for u in range(1, 6) for k in (2, 5)}
                      | {58})
            _p3_batch(nc, cx, qT1, kT1, vT1, XT1, SQ1, filler=fill,
                      fill_steps=fsteps)
            while fill:
                fill.popleft()()
            # Batch 1 epilogue (tail): interleave the qc0-dependent P5
            # chunks with the qc1 LN0 scalar chain (ACT/Pool roundtrips)
            # so the tail's serial DVE stream is the only remaining cost.
            st1 = _p5_alloc(nc, cx, XnT1)
            _p5_xbar(nc, cx, st1, 0)
            mu1, rstd1 = _p4_stats(nc, cx, XT1, SQ1, 1)
            for nqt in range(4):
                _p5_chunk(nc, cx, st1, nqt, relu_act=True)
            _p4_norm(nc, cx, XT1, ln0_aff, XnT1, 1, mu1, rstd1)
            _p5_xbar(nc, cx, st1, 1)
            _p5_finish_g(nc, cx, st1, dOut, NQ, ln1_aff, 0)
            for nqt in range(4, 8):
                _p5_chunk(nc, cx, st1, nqt, relu_act=True)
            _p5_finish_g(nc, cx, st1, dOut, NQ, ln1_aff, 1)

        if repeat == 1:
            body()
        else:
            # Branch hints: the body far exceeds one IRAM block per engine,
            # so the back-edge would I$-miss (~4us) without prefetch hints.
            hints = (mybir.EngineType.PE, mybir.EngineType.DVE,
                     mybir.EngineType.Activation, mybir.EngineType.Pool,
                     mybir.EngineType.SP)
            with tc.For_i(0, repeat, 1, hint_engines=hints):
                body()

    nc.compile()
    return nc


def _consts(Wq, Wk, Wv, Wo, flags, bq, bk, bv, bo, g0, b0, g1, b1):
    (bq_nz, bk_nz, bv_nz, bo_nz, ln0_aff, ln1_aff) = flags
    c = {
        "Wqb": np.ascontiguousarray(np.asarray(Wq).astype(NBF)),
        "Wkb": np.ascontiguousarray(np.asarray(Wk).astype(NBF)),
        "Wvb": np.ascontiguousarray(np.asarray(Wv).astype(NBF)),
        "Wob": np.ascontiguousarray(np.asarray(Wo).astype(NBF)),
        "onesc": np.ones((128, 1), NBF),
        "ones64": np.ones((128, 64), NBF),
    }
    if bq_nz: c["bq4"] = np.ascontiguousarray(np.asarray(bq).reshape(4, 128).T.astype(np.float32))
    if bk_nz: c["bk4"] = np.ascontiguousarray(np.asarray(bk).reshape(4, 128).T.astype(np.float32))
    if bv_nz: c["bvb"] = np.ascontiguousarray(np.broadcast_to(np.asarray(bv, np.float32), (128, D)))
    if bo_nz: c["bob"] = np.ascontiguousarray(np.broadcast_to(np.asarray(bo, np.float32), (128, D)))
    if ln0_aff:
        c["g04"] = np.ascontiguousarray(np.asarray(g0).reshape(4, 128).T.astype(np.float32))
        c["b04"] = np.ascontiguousarray(np.asarray(b0).reshape(4, 128).T.astype(np.float32))
    if ln1_aff:
        c["g1b"] = np.ascontiguousarray(np.broadcast_to(np.asarray(g1, np.float32), (128, D)))
        c["b1b"] = np.ascontiguousarray(np.broadcast_to(np.asarray(b1, np.float32), (128, D)))
    return c


def make_in_maps(Q, K, Wq, bq, Wk, bk, Wv, bv, Wo, bo, g0, b0, g1, b1, flags):
    consts = _consts(Wq, Wk, Wv, Wo, flags, bq, bk, bv, bo, g0, b0, g1, b1)
    in_maps = []
    for ci in range(N_CORES):
        m = dict(consts)
        m["Qs"] = np.ascontiguousarray(
            np.asarray(Q)[ci * BL:(ci + 1) * BL].reshape(BL * NQ, D).astype(NBF))
        m["Ks"] = np.ascontiguousarray(
            np.asarray(K)[ci * BL:(ci + 1) * BL].reshape(BL * NK, D).astype(NBF))
        in_maps.append(m)
    return in_maps


def get_flags(bq, bk, bv, bo, g0, b0, g1, b1):
    return (bool(np.any(np.asarray(bq))), bool(np.any(np.asarray(bk))),
            bool(np.any(np.asarray(bv))), bool(np.any(np.asarray(bo))),
            bool(np.any(np.asarray(g0) != 1) or np.any(np.asarray(b0))),
            bool(np.any(np.asarray(g1) != 1) or np.any(np.asarray(b1))))


def get_program(flags, repeat=1):
    key = (flags, repeat)
    if key not in _cache:
        _cache[key] = _build(flags, repeat)
    return _cache[key]


def kernel(Q, K, Wq, bq, Wk, bk, Wv, bv, Wo, bo, g0, b0, g1, b1):
    flags = get_flags(bq, bk, bv, bo, g0, b0, g1, b1)
    nc = get_program(flags, repeat=1)
    in_maps = make_in_maps(Q, K, Wq, bq, Wk, bk, Wv, bv, Wo, bo, g0, b0, g1, b1, flags)
    res = run_bass_kernel_spmd(nc, in_maps, list(range(N_CORES)))
    out = np.empty((B, NQ, D), np.float32)
    for ci in range(N_CORES):
        out[ci * BL:(ci + 1) * BL] = res.results[ci]["OUT"].reshape(BL, NQ, D)
    return out
